# revision 2
# baseline (speedup 1.0000x reference)
"""Bass/Trainium2 kernel for nn_BuildLstmUnrollNet — fp8 DoubleRow version.

Problem: 2-layer LSTM, unrolled T=11 steps with per-step (non-shared)
weights, B=8192, R=425, IN=20.  Output block t is the last-layer h
*before* step t, so only steps 0..9 need computing.

Strategy (data-parallel over batch, 8 cores x 1024 rows):
  - Gates are computed batch-major with the *transposed activations*
    stationary (lhsT) in fp8e4 and the weights moving in fp8e4, using
    perf_mode=DoubleRow (2 fp8 MACs/cell/cycle -> 2x PE throughput).
  - Error compensation to stay inside the 2e-2 gate:
      * weights split hi+lo (two fp8 planes, effective ~11-bit weights)
      * activations: fp8-hi everywhere plus fp8-lo "compensation rows"
        for 256 of the 425 h rows per tensor (h0, h0', h1), carried as
        extra K-chunks whose weight blocks are the hi planes.
    Per m-tile per step: layer0 = 5 DoubleRow passes, layer1 = 9.
  - The recurrent transpose h -> hT runs directly SBUF->SBUF on the
    xbar (bf16), no DRAM bounce; DVE converts the transposed bf16 to
    the fp8 hi plane and computes the fp8 lo plane (hi-lo subtract).
  - Cell math in bf16 on DVE (2x mode), c kept bf16; one fused sigmoid
    over i|f|o + tanh(g) straight out of PSUM on ACT; tanh(c) merged
    over m-tile pairs.

kernel(**inputs) takes full-size numpy inputs, does the host-side
packing/sharding, runs the same program SPMD on cores 0..7, and
reassembles the full [8192, 4675] fp32 output (block 0 comes straight
from the initial state on the host).
"""

import numpy as np
import ml_dtypes

BF16 = ml_dtypes.bfloat16
F8 = ml_dtypes.float8_e4m3fn

B = 8192
NCORES = 8
BC = B // NCORES          # batch rows per core (1024)
NB = BC // 128            # m-tiles per core (8)
R = 425
IN = 20
GN = 4 * R                # 1700 gate columns
GNP = 1712                # 16-aligned weight group stride
H1OFF = R + 1 + IN        # 446: h1 col offset in the packed state block
HC = 896                  # packed state block width (7*128)
NKC = HC // 128           # 7 hi chunks
NCH = 11                  # 7 hi + 4 lo chunks (7=lo c0, 8=lo c1, 9=lo c4, 10=lo c5)
NSTEPS = 10
NCHUNKS = [(0, 512), (512, 512), (1024, 512), (1536, 164)]

# Pass plans: (chunkA, chunkB, wsrcA, wsrcB); wsrc = (plane, chunk) with
# plane in {h0,l0} (layer-0 hi/lo) or {h1,l1} (layer-1 hi/lo).
L0P = [
    (0, 1, ("h0", 0), ("h0", 1)),
    (2, 3, ("h0", 2), ("h0", 3)),
    (0, 1, ("l0", 0), ("l0", 1)),
    (2, 3, ("l0", 2), ("l0", 3)),
    (7, 8, ("h0", 0), ("h0", 1)),      # act-comp h0 rows 0..255
]
L1P = [
    (0, 1, ("h1", 0), ("h1", 1)),
    (2, 3, ("h1", 2), ("h1", 3)),
    (4, 5, ("h1", 4), ("h1", 5)),
    (6, 7, ("h1", 6), ("h1", 0)),      # + act-comp h0' rows 0..127
    (0, 1, ("l1", 0), ("l1", 1)),
    (2, 3, ("l1", 2), ("l1", 3)),
    (4, 5, ("l1", 4), ("l1", 5)),
    (6, 8, ("l1", 6), ("h1", 1)),      # + act-comp h0' rows 128..255
    (9, 10, ("h1", 4), ("h1", 5)),     # act-comp h1 rows 66..321
]
NPASS = len(L0P) + len(L1P)            # 14

# set by test.py to profile; results stashed in LAST_RESULT
TRACE = False
LAST_RESULT = None


def build_bass(n_steps=NSTEPS, finalize=True):
    import concourse.bacc as bacc
    import concourse.mybir as mybir
    import concourse.tile as tile

    f32 = mybir.dt.float32
    bf16 = mybir.dt.bfloat16
    f8 = mybir.dt.float8e4
    DR = mybir.MatmulPerfMode.DoubleRow
    Sig = mybir.ActivationFunctionType.Sigmoid
    Tanh = mybir.ActivationFunctionType.Tanh

    nc = bacc.Bacc()

    w_d = nc.declare_dram_parameter("w", [n_steps, 128, NPASS * 2 * GNP], f8,
                                    False)
    hci_d = nc.declare_dram_parameter("hci", [128, NB * HC], bf16, False)
    htci_d = nc.declare_dram_parameter("htci", [128, NKC * BC], bf16, False)
    c0i_d = nc.declare_dram_parameter("c0i", [128, NB * R], bf16, False)
    c1i_d = nc.declare_dram_parameter("c1i", [128, NB * R], bf16, False)
    # out[p, m, t*R+r] = h1 block t for batch row m*128+p (host reassembles)
    out_d = nc.declare_dram_parameter("out", [128, NB, n_steps * R], f32, True)

    with tile.TileContext(nc) as tc:
        with (
            tc.tile_pool(name="consts", bufs=1) as consts,
            tc.tile_pool(name="wpool", bufs=2) as wpool,
            tc.tile_pool(name="gpsum", bufs=2, space="PSUM") as gpsum,
            tc.tile_pool(name="tmp", bufs=3) as tmp,
        ):
            # persistent state tiles
            hs_t = consts.tile([128, NB * HC], bf16)   # packed batch-major
            htc = consts.tile([128, NKC, BC], bf16)    # transposed, bf16
            htc8 = consts.tile([128, NCH, BC], f8)     # fp8 hi + lo chunks
            c0 = consts.tile([128, NB * R], bf16)
            c1 = consts.tile([128, NB * R], bf16)
            h1f = consts.tile([128, NB, R], f32)       # fp32 h1 for output

            # init DMAs on the SP (HWDGE) queue while the first weight
            # chunks stream on the Pool (SWDGE) queue
            for k in range(NKC):
                nc.sync.dma_start(htc[:, k, :], htci_d[:, k * BC:(k + 1) * BC])
            nc.sync.dma_start(c0[:], c0i_d[:])
            nc.sync.dma_start(hs_t[:], hci_d[:])
            nc.sync.dma_start(c1[:], c1i_d[:])

            # step-0 weights, split per pass-pair so matmuls start early
            w = wpool.tile([128, NPASS * 2, GNP], f8, tag="w")
            for p in range(NPASS):
                nc.gpsimd.dma_start(
                    w[:, 2 * p: 2 * p + 2, :],
                    w_d[0][:, 2 * p * GNP: (2 * p + 2) * GNP])

            # initial fp8 conversion of the transposed state
            nc.vector.tensor_copy(htc8[:, 0:NKC, :], htc[:, 0:NKC, :])
            nc.vector.tensor_sub(htc8[:, 7:9, :], htc[:, 0:2, :],
                                 htc8[:, 0:2, :])
            nc.vector.tensor_sub(htc8[:, 9:11, :], htc[:, 4:6, :],
                                 htc8[:, 4:6, :])

            # PE warm-up: the HAM clock gate needs ~3.4us of sustained
            # activity before the PE runs at full rate.
            warm = consts.tile([128, 128], bf16)
            nc.vector.memset(warm[:], 0.0)
            wps = gpsum.tile([128, 512], f32, tag="g")
            for i in range(20):
                nc.tensor.matmul(wps[:, 0:128], warm[:], warm[:],
                                 start=True, stop=True)

            # m-tile groups: transposes fire after the last m of a group
            GROUPS = [(0, 4), (4, 6), (6, 8)]

            def mgroup(m):
                for gi, (lo, hi) in enumerate(GROUPS):
                    if m == hi - 1:
                        return gi, lo, hi
                return None

            for t in range(n_steps):
                if t < n_steps - 1:
                    w_next = wpool.tile([128, NPASS * 2, GNP], f8, tag="w")
                    for c in range(4):
                        plo = c * 4 * 2 * GNP
                        phi = min((c + 1) * 4 * 2 * GNP, NPASS * 2 * GNP)
                        nc.gpsimd.dma_start(
                            w_next[:, 2 * c * 4: min(2 * (c + 1) * 4,
                                                     2 * NPASS), :],
                            w_d[t + 1][:, plo: phi])

                if t > 0:
                    # h1 chunks (4..6) were transposed at the end of step
                    # t-1; produce their fp8 hi planes + h1 lo chunks now
                    # (layer 0 below only touches chunks 0..3/7/8)
                    nc.vector.tensor_copy(htc8[:, 4:7, :], htc[:, 4:7, :])
                    nc.vector.tensor_sub(htc8[:, 9:11, :], htc[:, 4:6, :],
                                         htc8[:, 4:6, :])

                for layer in range(2):
                    passes = L0P if layer == 0 else L1P
                    poff = 0 if layer == 0 else len(L0P)
                    nk = len(passes)
                    cst = c0 if layer == 0 else c1
                    for m in range(NB):
                        g = gpsum.tile([128, GN], f32, tag="g")
                        for ki, (ca, cb, _, _) in enumerate(passes):
                            st = cb - ca
                            lhsT = htc8[:, ca:cb + 1:st,
                                        m * 128:(m + 1) * 128]
                            p = poff + ki
                            for (no, nw) in NCHUNKS:
                                nc.tensor.matmul(
                                    g[:, no: no + nw],
                                    lhsT,
                                    w[:, 2 * p: 2 * p + 2, no: no + nw],
                                    start=(ki == 0),
                                    stop=(ki == nk - 1),
                                    perf_mode=DR,
                                )

                        # LSTM cell (torch gate order: i, f, o, g), bf16
                        cs = cst[:, m * R:(m + 1) * R]
                        tsig = tmp.tile([128, 3 * R], bf16, tag="tsig")
                        nc.scalar.activation(tsig[:], g[:, 0:3 * R], Sig)
                        tg = tmp.tile([128, R], bf16, tag="tg")
                        nc.scalar.activation(tg[:], g[:, 3 * R:4 * R], Tanh)

                        tig = tmp.tile([128, R], bf16, tag="tig")
                        nc.vector.tensor_mul(tig[:], tsig[:, 0:R], tg[:])
                        tfc = tmp.tile([128, R], bf16, tag="tfc")
                        nc.vector.tensor_mul(tfc[:], tsig[:, R:2 * R], cs)
                        nc.vector.tensor_add(cs, tfc[:], tig[:])
                        ttc = tmp.tile([128, R], bf16, tag="ttc")
                        nc.scalar.activation(ttc[:], cs, Tanh)

                        if layer == 0:
                            # h0' -> packed bf16 cols 0..424 (2x DVE)
                            nc.vector.tensor_mul(
                                hs_t[:, m * HC: m * HC + R],
                                tsig[:, 2 * R:3 * R], ttc[:])
                        else:
                            hh = h1f[:, m, :]
                            nc.vector.tensor_mul(hh, tsig[:, 2 * R:3 * R],
                                                 ttc[:])
                            if t < n_steps - 1:
                                nc.vector.tensor_copy(
                                    hs_t[:, m * HC + H1OFF:
                                         m * HC + H1OFF + R], hh)

                        grp = mgroup(m)
                        if grp is None:
                            continue
                        _, mlo, mhi = grp
                        cols = slice(mlo * 128, mhi * 128)
                        if layer == 0:
                            # transpose this group's h0'|1|x|h1head cols,
                            # then fp8 hi planes + h0' lo chunks
                            for mm in range(mlo, mhi):
                                nc.sync.dma_start(
                                    htc[:, 0:4, mm * 128:(mm + 1) * 128],
                                    hs_t[:, mm * HC: mm * HC + 512],
                                    transpose=True)
                            nc.vector.tensor_copy(htc8[:, 0:4, cols],
                                                  htc[:, 0:4, cols])
                            nc.vector.tensor_sub(htc8[:, 7:9, cols],
                                                 htc[:, 0:2, cols],
                                                 htc8[:, 0:2, cols])
                        elif t < n_steps - 1:
                            # transpose the group's h1 cols (chunks 4..6);
                            # fp8 conversion happens at the top of step t+1
                            for mm in range(mlo, mhi):
                                nc.sync.dma_start(
                                    htc[:, 4:7, mm * 128:(mm + 1) * 128],
                                    hs_t[:, mm * HC + 512:(mm + 1) * HC],
                                    transpose=True)

                # one bulk output store per step on the Pool queue
                nc.gpsimd.dma_start(
                    out_d[:, :, t * R:(t + 1) * R], h1f[:, :, :])
                if t < n_steps - 1:
                    w = w_next
    if finalize:
        nc.finalize()
    return nc


def _pack_pf(a):
    """[BC, C] -> [128, NB*C] with m-tile m at cols m*C."""
    c = a.shape[1]
    return np.ascontiguousarray(
        a.reshape(NB, 128, c).transpose(1, 0, 2).reshape(128, NB * c))


def _pack_kt(a):
    """[BC, HC] -> transposed [128, NKC*BC] with K-chunk k at cols k*BC."""
    return np.ascontiguousarray(
        a.T.reshape(NKC, 128, BC).transpose(1, 0, 2).reshape(128, NKC * BC))


def _q8(a):
    return a.astype(F8).astype(np.float32)


def prep_inputs(x, init_states_input, W_i2h0, b_i2h0, W_h2h0, b_h2h0,
                W_i2h1, b_i2h1, W_h2h1, b_h2h1, n_steps=NSTEPS):
    """Host-side packing.  Returns (in_maps, h1_init_full)."""
    x = np.asarray(x, np.float32)
    init = np.asarray(init_states_input, np.float32)
    W_i2h0 = np.asarray(W_i2h0, np.float32)
    b_i2h0 = np.asarray(b_i2h0, np.float32)
    W_h2h0 = np.asarray(W_h2h0, np.float32)
    b_h2h0 = np.asarray(b_h2h0, np.float32)
    W_i2h1 = np.asarray(W_i2h1, np.float32)
    b_i2h1 = np.asarray(b_i2h1, np.float32)
    W_h2h1 = np.asarray(W_h2h1, np.float32)
    b_h2h1 = np.asarray(b_h2h1, np.float32)

    # per-step packed-row weight planes, then per-pass fp8 blocks
    w_dev = np.zeros((n_steps, 128, NPASS * 2 * GNP), F8)
    for t in range(n_steps):
        L0row = np.zeros((512, GN), np.float32)
        L0row[0:R] = W_h2h0[t].T
        L0row[R] = b_i2h0[t] + b_h2h0[t]
        L0row[R + 1: R + 1 + IN] = W_i2h0[t].T
        L1row = np.zeros((HC, GN), np.float32)
        L1row[0:R] = W_i2h1[t].T
        L1row[R] = b_i2h1[t] + b_h2h1[t]
        L1row[H1OFF: H1OFF + R] = W_h2h1[t].T

        L0hi = _q8(L0row)
        L0lo = _q8(L0row - L0hi)
        L1hi = _q8(L1row)
        L1lo = _q8(L1row - L1hi)
        planes = {"h0": L0hi, "l0": L0lo, "h1": L1hi, "l1": L1lo}

        Wt = np.zeros((NPASS, 2, 128, GNP), np.float32)
        for p, (_, _, wa, wb) in enumerate(L0P + L1P):
            for gsel, (plane, ck) in ((0, wa), (1, wb)):
                Wt[p, gsel, :, 0:GN] = planes[plane][ck * 128:(ck + 1) * 128]
        # -> [128, NPASS*2*GNP]
        w_dev[t] = np.ascontiguousarray(
            Wt.transpose(2, 0, 1, 3).reshape(128, NPASS * 2 * GNP)).astype(F8)

    init4 = init.reshape(B, 4, R)
    h0_full, c0_full = init4[:, 0], init4[:, 1]
    h1_full, c1_full = init4[:, 2], init4[:, 3]

    in_maps = []
    for c in range(NCORES):
        sl = slice(c * BC, (c + 1) * BC)
        hcp = np.zeros((BC, HC), np.float32)
        hcp[:, 0:R] = h0_full[sl]
        hcp[:, R] = 1.0
        hcp[:, R + 1: R + 1 + IN] = x[sl]
        hcp[:, H1OFF: H1OFF + R] = h1_full[sl]
        hcp = hcp.astype(BF16)
        in_maps.append({
            "w": w_dev,
            "hci": _pack_pf(hcp),
            "htci": _pack_kt(hcp),
            "c0i": _pack_pf(np.ascontiguousarray(c0_full[sl])).astype(BF16),
            "c1i": _pack_pf(np.ascontiguousarray(c1_full[sl])).astype(BF16),
        })
    return in_maps, h1_full


def kernel(x, init_states_input, W_i2h0, b_i2h0, W_h2h0, b_h2h0,
           W_i2h1, b_i2h1, W_h2h1, b_h2h1):
    global LAST_RESULT
    from concourse.bass_utils import run_bass_kernel_spmd

    in_maps, h1_full = prep_inputs(
        x, init_states_input, W_i2h0, b_i2h0, W_h2h0, b_h2h0,
        W_i2h1, b_i2h1, W_h2h1, b_h2h1)

    nc = build_bass(NSTEPS)
    res = run_bass_kernel_spmd(nc, in_maps, list(range(NCORES)), trace=TRACE)
    LAST_RESULT = res

    out = np.empty((B, (NSTEPS + 1) * R), np.float32)
    out[:, 0:R] = h1_full
    for c in range(NCORES):
        # device out[p, m, :] = batch row m*128+p
        dev = res.results[c]["out"]
        out[c * BC:(c + 1) * BC, R:] = (
            dev.transpose(1, 0, 2).reshape(BC, NSTEPS * R))
    return out


# revision 24
# speedup vs baseline: 1.1367x; 1.1367x over previous
"""Bass/Trainium2 kernel for nn_BuildLstmUnrollNet — fp8 DoubleRow version.

Problem: 2-layer LSTM, unrolled T=11 steps with per-step (non-shared)
weights, B=8192, R=425, IN=20.  Output block t is the last-layer h
*before* step t, so only steps 0..9 need computing.

Strategy (data-parallel over batch, 8 cores x 1024 rows):
  - Gates are computed batch-major with the *transposed activations*
    stationary (lhsT) in fp8e4 and the weights moving in fp8e4, using
    perf_mode=DoubleRow (2 fp8 MACs/cell/cycle -> 2x PE throughput).
  - Error compensation to stay inside the 2e-2 gate:
      * weights split hi+lo (two fp8 planes, effective ~11-bit weights)
      * activations: fp8-hi everywhere plus fp8-lo "compensation rows"
        for 256 of the 425 h rows per tensor (h0, h0', h1), carried as
        extra K-chunks whose weight blocks are the hi planes.
    Per m-tile per step: layer0 = 5 DoubleRow passes, layer1 = 9.
  - The recurrent transpose h -> hT runs directly SBUF->SBUF on the
    xbar (bf16), no DRAM bounce; DVE converts the transposed bf16 to
    the fp8 hi plane and computes the fp8 lo plane (hi-lo subtract).
  - Cell math in bf16 on DVE (2x mode), c kept bf16; one fused sigmoid
    over i|f|o + tanh(g) straight out of PSUM on ACT; tanh(c) merged
    over m-tile pairs.

kernel(**inputs) takes full-size numpy inputs, does the host-side
packing/sharding, runs the same program SPMD on cores 0..7, and
reassembles the full [8192, 4675] fp32 output (block 0 comes straight
from the initial state on the host).
"""

import numpy as np
import ml_dtypes

BF16 = ml_dtypes.bfloat16
F8 = ml_dtypes.float8_e4m3fn

B = 8192
NCORES = 8
BC = B // NCORES          # batch rows per core (1024)
NB = BC // 128            # m-tiles per core (8)
R = 425
IN = 20
GN = 4 * R                # 1700 gate columns
GNP = 1712                # 16-aligned weight group stride
H1OFF = R + 1 + IN        # 446: h1 col offset in the packed state block
HC = 896                  # packed state block width (7*128)
NKC = HC // 128           # 7 hi chunks
NCH = 11                  # 7 hi + 4 lo chunks (7=lo c0, 8=lo c1, 9=lo c4, 10=lo c5)
NSTEPS = 10
NCHUNKS = [(0, 512), (512, 512), (1024, 512), (1536, 164)]

# Pass plans: (chunkA, chunkB, wsrcA, wsrcB); wsrc = (plane, chunk) with
# plane in {h0,l0} (layer-0 hi/lo) or {h1,l1} (layer-1 hi/lo).
L0P = [
    (0, 1, ("h0", 0), ("h0", 1)),
    (2, 3, ("h0", 2), ("h0", 3)),
    (0, 1, ("l0", 0), ("l0", 1)),
    (2, 3, ("l0", 2), ("l0", 3)),
]
# h0'-only passes first, h1-dependent chunks (3 is mid-step, 4..6,9,10 are
# end-of-previous-step) last, so layer-1 matmuls start before the h1
# fp8 planes for this step are finished
L1P = [
    (0, 1, ("h1", 0), ("h1", 1)),
    (2, 3, ("h1", 2), ("h1", 3)),
    (0, 1, ("l1", 0), ("l1", 1)),
    (2, 3, ("l1", 2), ("l1", 3)),
    (6, 7, ("h1", 6), ("h1", 0)),      # + act-comp h0' rows 0..127
    (6, 8, ("l1", 6), ("h1", 1)),      # + act-comp h0' rows 128..255
    (4, 5, ("h1", 4), ("h1", 5)),
    (4, 5, ("l1", 4), ("l1", 5)),
    (9, 10, ("h1", 4), ("h1", 5)),     # act-comp h1 rows 66..321
]
NPASS = len(L0P) + len(L1P)            # 14

# set by test.py to profile; results stashed in LAST_RESULT
TRACE = False
LAST_RESULT = None


def build_bass(n_steps=NSTEPS, finalize=True):
    import concourse.bacc as bacc
    import concourse.mybir as mybir
    import concourse.tile as tile

    f32 = mybir.dt.float32
    bf16 = mybir.dt.bfloat16
    f8 = mybir.dt.float8e4
    DR = mybir.MatmulPerfMode.DoubleRow
    Sig = mybir.ActivationFunctionType.Sigmoid
    Tanh = mybir.ActivationFunctionType.Tanh

    nc = bacc.Bacc()

    w_d = nc.declare_dram_parameter("w", [n_steps, 128, NPASS * 2 * GNP], f8,
                                    False)
    hci_d = nc.declare_dram_parameter("hci", [128, NB * HC], bf16, False)
    htci_d = nc.declare_dram_parameter("htci", [128, NKC * BC], bf16, False)
    c0i_d = nc.declare_dram_parameter("c0i", [128, NB * R], bf16, False)
    c1i_d = nc.declare_dram_parameter("c1i", [128, NB * R], bf16, False)
    # out[p, m, t*R+r] = h1 block t for batch row m*128+p (host reassembles)
    out_d = nc.declare_dram_parameter("out", [128, NB, n_steps * R], f32, True)

    with tile.TileContext(nc) as tc:
        with (
            tc.tile_pool(name="consts", bufs=1) as consts,
            tc.tile_pool(name="wpool", bufs=2) as wpool,
            tc.tile_pool(name="gpsum", bufs=2, space="PSUM") as gpsum,
            tc.tile_pool(name="tmp", bufs=4) as tmp,
            tc.tile_pool(name="h1pool", bufs=2) as h1pool,
        ):
            # persistent state tiles
            hs_t = consts.tile([128, NB * HC], bf16)   # packed batch-major
            htc = consts.tile([128, NKC, BC], bf16)    # transposed, bf16
            htc8 = consts.tile([128, NCH, BC], f8)     # fp8 hi + lo chunks
            c0 = consts.tile([128, NB * R], bf16)
            c1 = consts.tile([128, NB * R], bf16)

            # init DMAs on the SP (HWDGE) queue while the first weight
            # chunks stream on the Pool (SWDGE) queue
            for k in range(NKC):
                nc.sync.dma_start(htc[:, k, :], htci_d[:, k * BC:(k + 1) * BC])
            nc.sync.dma_start(c0[:], c0i_d[:])
            nc.sync.dma_start(hs_t[:], hci_d[:])
            nc.sync.dma_start(c1[:], c1i_d[:])

            # step-0 weights, split per pass-pair so matmuls start early
            w = wpool.tile([128, NPASS * 2, GNP], f8, tag="w")
            for p in range(NPASS):
                nc.gpsimd.dma_start(
                    w[:, 2 * p: 2 * p + 2, :],
                    w_d[0][:, 2 * p * GNP: (2 * p + 2) * GNP])

            # initial fp8 conversion of the transposed state
            nc.vector.tensor_copy(htc8[:, 0:NKC, :], htc[:, 0:NKC, :])
            nc.vector.tensor_sub(htc8[:, 7:9, :], htc[:, 0:2, :],
                                 htc8[:, 0:2, :])
            nc.vector.tensor_sub(htc8[:, 9:11, :], htc[:, 4:6, :],
                                 htc8[:, 4:6, :])

            # PE warm-up: the HAM clock gate needs ~3.4us of sustained
            # activity before the PE runs at full rate.
            warm = consts.tile([128, 128], bf16)
            nc.vector.memset(warm[:], 0.0)
            wps = gpsum.tile([128, 512], f32, tag="g")
            for i in range(20):
                nc.tensor.matmul(wps[:, 0:128], warm[:], warm[:],
                                 start=True, stop=True)



            for t in range(n_steps):
                h1f = h1pool.tile([128, NB, R], f32, tag="h1f")
                if t > 0:
                    # h1 chunks (4..6) were transposed at the end of step
                    # t-1; produce their fp8 hi planes + h1 lo chunks on
                    # GPSIMD so neither the DVE nor the ACT in-order
                    # queues ever wait on them.  Layer 1's h1 passes (the
                    # last 5 of L1P) are the only consumers.  These go
                    # FIRST on the Pool queue: the w prefetch below has a
                    # WAR wait on step t-1's matmuls and a step of slack.
                    nc.gpsimd.tensor_copy(htc8[:, 6:7, :], htc[:, 6:7, :])
                    nc.gpsimd.tensor_copy(htc8[:, 4:6, :], htc[:, 4:6, :])
                    nc.gpsimd.tensor_sub(htc8[:, 9:11, :], htc[:, 4:6, :],
                                         htc8[:, 4:6, :])

                for layer in range(2):
                    passes = L0P if layer == 0 else L1P
                    poff = 0 if layer == 0 else len(L0P)
                    nk = len(passes)
                    cst = c0 if layer == 0 else c1
                    osig = [None] * NB   # per-m tsig kept for finishB()
                    ottc = [None] * NB   # per-m tanh(c) kept for finishB()

                    def finishA(m, cst):
                        """tanh(c) for cell m — emitted after cell m+1's
                        sig/tanh so the ACT queue never waits on DVE."""
                        cs = cst[:, m * R:(m + 1) * R]
                        ttc = tmp.tile([128, R], bf16, tag="ttc")
                        nc.scalar.activation(ttc[:], cs, Tanh)
                        ottc[m] = ttc

                    def finishB(m, layer, t):
                        """h-mul + transpose/fp8 for cell m — trails by 2
                        cells so the DVE queue never waits on tanh(c)."""
                        tsig = osig[m]
                        ttc = ottc[m]
                        if layer == 0:
                            nc.vector.tensor_mul(
                                hs_t[:, m * HC: m * HC + R],
                                tsig[:, 2 * R:3 * R], ttc[:])
                        else:
                            hh = h1f[:, m, :]
                            nc.vector.tensor_mul(hh, tsig[:, 2 * R:3 * R],
                                                 ttc[:])
                            if t < n_steps - 1:
                                nc.vector.tensor_copy(
                                    hs_t[:, m * HC + H1OFF:
                                         m * HC + H1OFF + R], hh)
                        cols = slice(m * 128, (m + 1) * 128)
                        if layer == 0:
                            # transpose THIS m-tile's h0'|1|x|h1head cols,
                            # then its fp8 hi planes + h0' lo chunks —
                            # layer 1's m-tile m only reads its own 128
                            # columns, so it unblocks immediately
                            nc.sync.dma_start(
                                htc[:, 0:4, cols],
                                hs_t[:, m * HC: m * HC + 512],
                                transpose=True)
                            eng = nc.vector if m < 2 else nc.gpsimd
                            eng.tensor_copy(htc8[:, 0:4, cols],
                                            htc[:, 0:4, cols])
                            eng.tensor_sub(htc8[:, 7:9, cols],
                                           htc[:, 0:2, cols],
                                           htc8[:, 0:2, cols])
                        elif t < n_steps - 1:
                            # this m-tile's h1 cols (chunks 4..6); their
                            # fp8 planes are made at the top of step t+1
                            nc.sync.dma_start(
                                htc[:, 4:7, cols],
                                hs_t[:, m * HC + 512:(m + 1) * HC],
                                transpose=True)

                    for m in range(NB):
                        g = gpsum.tile([128, GN], f32, tag="g")
                        for ki, (ca, cb, _, _) in enumerate(passes):
                            st = cb - ca
                            lhsT = htc8[:, ca:cb + 1:st,
                                        m * 128:(m + 1) * 128]
                            p = poff + ki
                            for (no, nw) in NCHUNKS:
                                nc.tensor.matmul(
                                    g[:, no: no + nw],
                                    lhsT,
                                    w[:, 2 * p: 2 * p + 2, no: no + nw],
                                    start=(ki == 0),
                                    stop=(ki == nk - 1),
                                    perf_mode=DR,
                                )

                        # cell part A (torch gate order: i, f, o, g), bf16
                        cs = cst[:, m * R:(m + 1) * R]
                        tsig = tmp.tile([128, 3 * R], bf16, tag="tsig")
                        nc.scalar.activation(tsig[:], g[:, 0:3 * R], Sig)
                        tg = tmp.tile([128, R], bf16, tag="tg")
                        nc.scalar.activation(tg[:], g[:, 3 * R:4 * R], Tanh)
                        tig = tmp.tile([128, R], bf16, tag="tig")
                        nc.vector.tensor_mul(tig[:], tsig[:, 0:R], tg[:])
                        tfc = tmp.tile([128, R], bf16, tag="tfc")
                        nc.vector.tensor_mul(tfc[:], tsig[:, R:2 * R], cs)
                        nc.vector.tensor_add(cs, tfc[:], tig[:])
                        osig[m] = tsig

                        if m >= 1:
                            finishA(m - 1, cst)
                        if m >= 2:
                            finishB(m - 2, layer, t)
                    finishA(NB - 1, cst)
                    finishB(NB - 2, layer, t)
                    finishB(NB - 1, layer, t)

                # one bulk output store per step on the SP queue
                nc.sync.dma_start(
                    out_d[:, :, t * R:(t + 1) * R], h1f[:, :, :])
                if t < n_steps - 1:
                    # prefetch step t+1's weights (a full step of slack;
                    # emitted last so the Pool queue serves the mid-step
                    # fp8 converts first)
                    w_next = wpool.tile([128, NPASS * 2, GNP], f8, tag="w")
                    for c in range(7):
                        glo = c * 4
                        ghi = min((c + 1) * 4, 2 * NPASS)
                        nc.gpsimd.dma_start(
                            w_next[:, glo: ghi, :],
                            w_d[t + 1][:, glo * GNP: ghi * GNP])
                    w = w_next
    if finalize:
        nc.finalize()
    return nc


def _pack_pf(a):
    """[BC, C] -> [128, NB*C] with m-tile m at cols m*C."""
    c = a.shape[1]
    return np.ascontiguousarray(
        a.reshape(NB, 128, c).transpose(1, 0, 2).reshape(128, NB * c))


def _pack_kt(a):
    """[BC, HC] -> transposed [128, NKC*BC] with K-chunk k at cols k*BC."""
    return np.ascontiguousarray(
        a.T.reshape(NKC, 128, BC).transpose(1, 0, 2).reshape(128, NKC * BC))


def _q8(a):
    return a.astype(F8).astype(np.float32)


def prep_inputs(x, init_states_input, W_i2h0, b_i2h0, W_h2h0, b_h2h0,
                W_i2h1, b_i2h1, W_h2h1, b_h2h1, n_steps=NSTEPS):
    """Host-side packing.  Returns (in_maps, h1_init_full)."""
    x = np.asarray(x, np.float32)
    init = np.asarray(init_states_input, np.float32)
    W_i2h0 = np.asarray(W_i2h0, np.float32)
    b_i2h0 = np.asarray(b_i2h0, np.float32)
    W_h2h0 = np.asarray(W_h2h0, np.float32)
    b_h2h0 = np.asarray(b_h2h0, np.float32)
    W_i2h1 = np.asarray(W_i2h1, np.float32)
    b_i2h1 = np.asarray(b_i2h1, np.float32)
    W_h2h1 = np.asarray(W_h2h1, np.float32)
    b_h2h1 = np.asarray(b_h2h1, np.float32)

    # per-step packed-row weight planes, then per-pass fp8 blocks
    w_dev = np.zeros((n_steps, 128, NPASS * 2 * GNP), F8)
    for t in range(n_steps):
        L0row = np.zeros((512, GN), np.float32)
        L0row[0:R] = W_h2h0[t].T
        L0row[R] = b_i2h0[t] + b_h2h0[t]
        L0row[R + 1: R + 1 + IN] = W_i2h0[t].T
        L1row = np.zeros((HC, GN), np.float32)
        L1row[0:R] = W_i2h1[t].T
        L1row[R] = b_i2h1[t] + b_h2h1[t]
        L1row[H1OFF: H1OFF + R] = W_h2h1[t].T

        L0hi = _q8(L0row)
        L0lo = _q8(L0row - L0hi)
        L1hi = _q8(L1row)
        L1lo = _q8(L1row - L1hi)
        planes = {"h0": L0hi, "l0": L0lo, "h1": L1hi, "l1": L1lo}

        Wt = np.zeros((NPASS, 2, 128, GNP), np.float32)
        for p, (_, _, wa, wb) in enumerate(L0P + L1P):
            for gsel, (plane, ck) in ((0, wa), (1, wb)):
                Wt[p, gsel, :, 0:GN] = planes[plane][ck * 128:(ck + 1) * 128]
        # -> [128, NPASS*2*GNP]
        w_dev[t] = np.ascontiguousarray(
            Wt.transpose(2, 0, 1, 3).reshape(128, NPASS * 2 * GNP)).astype(F8)

    init4 = init.reshape(B, 4, R)
    h0_full, c0_full = init4[:, 0], init4[:, 1]
    h1_full, c1_full = init4[:, 2], init4[:, 3]

    in_maps = []
    for c in range(NCORES):
        sl = slice(c * BC, (c + 1) * BC)
        hcp = np.zeros((BC, HC), np.float32)
        hcp[:, 0:R] = h0_full[sl]
        hcp[:, R] = 1.0
        hcp[:, R + 1: R + 1 + IN] = x[sl]
        hcp[:, H1OFF: H1OFF + R] = h1_full[sl]
        hcp = hcp.astype(BF16)
        in_maps.append({
            "w": w_dev,
            "hci": _pack_pf(hcp),
            "htci": _pack_kt(hcp),
            "c0i": _pack_pf(np.ascontiguousarray(c0_full[sl])).astype(BF16),
            "c1i": _pack_pf(np.ascontiguousarray(c1_full[sl])).astype(BF16),
        })
    return in_maps, h1_full


def kernel(x, init_states_input, W_i2h0, b_i2h0, W_h2h0, b_h2h0,
           W_i2h1, b_i2h1, W_h2h1, b_h2h1):
    global LAST_RESULT
    from concourse.bass_utils import run_bass_kernel_spmd

    in_maps, h1_full = prep_inputs(
        x, init_states_input, W_i2h0, b_i2h0, W_h2h0, b_h2h0,
        W_i2h1, b_i2h1, W_h2h1, b_h2h1)

    nc = build_bass(NSTEPS)
    res = run_bass_kernel_spmd(nc, in_maps, list(range(NCORES)), trace=TRACE)
    LAST_RESULT = res

    out = np.empty((B, (NSTEPS + 1) * R), np.float32)
    out[:, 0:R] = h1_full
    for c in range(NCORES):
        # device out[p, m, :] = batch row m*128+p
        dev = res.results[c]["out"]
        out[c * BC:(c + 1) * BC, R:] = (
            dev.transpose(1, 0, 2).reshape(BC, NSTEPS * R))
    return out


# revision 32
# speedup vs baseline: 1.2219x; 1.0750x over previous
"""Bass/Trainium2 kernel for nn_BuildLstmUnrollNet — fp8 DoubleRow version.

Problem: 2-layer LSTM, unrolled T=11 steps with per-step (non-shared)
weights, B=8192, R=425, IN=20.  Output block t is the last-layer h
*before* step t, so only steps 0..9 need computing.

Strategy (data-parallel over batch, 8 cores x 1024 rows):
  - Gates are computed batch-major with the *transposed activations*
    stationary (lhsT) in fp8e4 and the weights moving in fp8e4, using
    perf_mode=DoubleRow (2 fp8 MACs/cell/cycle -> 2x PE throughput).
  - Error compensation to stay inside the 2e-2 gate:
      * weights split hi+lo (two fp8 planes, effective ~11-bit weights)
      * activations: fp8-hi everywhere plus fp8-lo "compensation rows"
        for 256 of the 425 h rows per tensor (h0, h0', h1), carried as
        extra K-chunks whose weight blocks are the hi planes.
    Per m-tile per step: layer0 = 5 DoubleRow passes, layer1 = 9.
  - The recurrent transpose h -> hT runs directly SBUF->SBUF on the
    xbar (bf16), no DRAM bounce; DVE converts the transposed bf16 to
    the fp8 hi plane and computes the fp8 lo plane (hi-lo subtract).
  - Cell math in bf16 on DVE (2x mode), c kept bf16; one fused sigmoid
    over i|f|o + tanh(g) straight out of PSUM on ACT; tanh(c) merged
    over m-tile pairs.

kernel(**inputs) takes full-size numpy inputs, does the host-side
packing/sharding, runs the same program SPMD on cores 0..7, and
reassembles the full [8192, 4675] fp32 output (block 0 comes straight
from the initial state on the host).
"""

import numpy as np
import ml_dtypes

BF16 = ml_dtypes.bfloat16
F8 = ml_dtypes.float8_e4m3fn

B = 8192
NCORES = 8
BC = B // NCORES          # batch rows per core (1024)
NB = BC // 128            # m-tiles per core (8)
R = 425
IN = 20
GN = 4 * R                # 1700 gate columns
GNP = 1712                # 16-aligned weight group stride
H1OFF = R + 1 + IN        # 446: h1 col offset in the packed state block
HC = 896                  # packed state block width (7*128)
NKC = HC // 128           # 7 hi chunks
NCH = 11                  # 7 hi + 4 lo chunks (7=lo c0, 8=lo c1, 9=lo c4, 10=lo c5)
NSTEPS = 10
NCHUNKS = [(0, 512), (512, 512), (1024, 512), (1536, 164)]

# Pass plans: (chunkA, chunkB, wsrcA, wsrcB); wsrc = (plane, chunk) with
# plane in {h0,l0} (layer-0 hi/lo) or {h1,l1} (layer-1 hi/lo).
L0P = [
    (0, 1, ("h0", 0), ("h0", 1)),
    (2, 3, ("h0", 2), ("h0", 3)),
    (0, 1, ("l0", 0), ("l0", 1)),
    (2, 3, ("l0", 2), ("l0", 3)),
]
# h0'-only passes first, h1-dependent chunks (3 is mid-step, 4..6,9,10 are
# end-of-previous-step) last, so layer-1 matmuls start before the h1
# fp8 planes for this step are finished
L1P = [
    (0, 1, ("h1", 0), ("h1", 1)),
    (2, 3, ("h1", 2), ("h1", 3)),
    (0, 1, ("l1", 0), ("l1", 1)),
    (2, 3, ("l1", 2), ("l1", 3)),
    (6, 7, ("h1", 6), ("h1", 0)),      # + act-comp h0' rows 0..127
    (6, 8, ("l1", 6), ("h1", 1)),      # + act-comp h0' rows 128..255
    (4, 5, ("h1", 4), ("h1", 5)),
    (4, 5, ("l1", 4), ("l1", 5)),
    (9, 10, ("h1", 4), ("h1", 5)),     # act-comp h1 rows 66..321
]
NPASS = len(L0P) + len(L1P)            # 14

# set by test.py to profile; results stashed in LAST_RESULT
TRACE = False
LAST_RESULT = None


def build_bass(n_steps=NSTEPS, finalize=True):
    import concourse.bacc as bacc
    import concourse.mybir as mybir
    import concourse.tile as tile

    f32 = mybir.dt.float32
    bf16 = mybir.dt.bfloat16
    f8 = mybir.dt.float8e4
    DR = mybir.MatmulPerfMode.DoubleRow
    Sig = mybir.ActivationFunctionType.Sigmoid
    Tanh = mybir.ActivationFunctionType.Tanh

    nc = bacc.Bacc()

    w_d = nc.declare_dram_parameter("w", [n_steps, 128, NPASS * 2 * GNP], f8,
                                    False)
    hci_d = nc.declare_dram_parameter("hci", [128, NB * HC], bf16, False)
    htci_d = nc.declare_dram_parameter("htci", [128, NKC * BC], bf16, False)
    c0i_d = nc.declare_dram_parameter("c0i", [128, NB * R], bf16, False)
    c1i_d = nc.declare_dram_parameter("c1i", [128, NB * R], bf16, False)
    # out[p, m, t*R+r] = h1 block t for batch row m*128+p (host reassembles)
    out_d = nc.declare_dram_parameter("out", [128, NB, n_steps * R], f32, True)

    with tile.TileContext(nc) as tc:
        with (
            tc.tile_pool(name="consts", bufs=1) as consts,
            tc.tile_pool(name="wpool", bufs=2) as wpool,
            tc.tile_pool(name="gpsum", bufs=2, space="PSUM") as gpsum,
            tc.tile_pool(name="tmp", bufs=4) as tmp,
            tc.tile_pool(name="h1pool", bufs=2) as h1pool,
        ):
            # persistent state tiles
            hs_t = consts.tile([128, NB * HC], bf16)   # packed batch-major
            htc = consts.tile([128, NKC, BC], bf16)    # transposed, bf16
            htc8 = consts.tile([128, NCH, BC], f8)     # fp8 hi + lo chunks
            c0 = consts.tile([128, NB * R], bf16)
            c1 = consts.tile([128, NB * R], bf16)

            # init DMAs on the SP (HWDGE) queue while the first weight
            # chunks stream on the Pool (SWDGE) queue
            for k in range(NKC):
                nc.sync.dma_start(htc[:, k, :], htci_d[:, k * BC:(k + 1) * BC])
            nc.sync.dma_start(c0[:], c0i_d[:])
            nc.sync.dma_start(hs_t[:], hci_d[:])
            nc.sync.dma_start(c1[:], c1i_d[:])

            # step-0 weights, split per pass-pair so matmuls start early
            w = wpool.tile([128, NPASS * 2, GNP], f8, tag="w")
            for p in range(NPASS):
                nc.gpsimd.dma_start(
                    w[:, 2 * p: 2 * p + 2, :],
                    w_d[0][:, 2 * p * GNP: (2 * p + 2) * GNP])

            # initial fp8 conversion of the transposed state
            nc.vector.tensor_copy(htc8[:, 0:NKC, :], htc[:, 0:NKC, :])
            nc.vector.tensor_sub(htc8[:, 7:9, :], htc[:, 0:2, :],
                                 htc8[:, 0:2, :])
            nc.vector.tensor_sub(htc8[:, 9:11, :], htc[:, 4:6, :],
                                 htc8[:, 4:6, :])

            # PE warm-up: the HAM clock gate needs ~3.4us of sustained
            # activity before the PE runs at full rate.
            warm = consts.tile([128, 128], bf16)
            nc.vector.memset(warm[:], 0.0)
            wps = gpsum.tile([128, 512], f32, tag="g")
            for i in range(20):
                nc.tensor.matmul(wps[:, 0:128], warm[:], warm[:],
                                 start=True, stop=True)



            for t in range(n_steps):
                h1f = h1pool.tile([128, NB, R], f32, tag="h1f")
                if t > 0:
                    # h1 chunks (4..6) were transposed at the end of step
                    # t-1; produce their fp8 hi planes + h1 lo chunks on
                    # GPSIMD so neither the DVE nor the ACT in-order
                    # queues ever wait on them.  Layer 1's h1 passes (the
                    # last 5 of L1P) are the only consumers.  These go
                    # FIRST on the Pool queue: the w prefetch below has a
                    # WAR wait on step t-1's matmuls and a step of slack.
                    nc.gpsimd.tensor_copy(htc8[:, 6:7, :], htc[:, 6:7, :])
                    nc.gpsimd.tensor_copy(htc8[:, 4:6, :], htc[:, 4:6, :])
                    nc.gpsimd.tensor_sub(htc8[:, 9:11, :], htc[:, 4:6, :],
                                         htc8[:, 4:6, :])

                for layer in range(2):
                    passes = L0P if layer == 0 else L1P
                    poff = 0 if layer == 0 else len(L0P)
                    nk = len(passes)
                    cst = c0 if layer == 0 else c1
                    osig = [None] * NB   # per-m tsig kept for finishB()
                    ottc = [None] * NB   # per-m tanh(c) kept for finishB()

                    def finishA(m, cst):
                        """tanh(c) for cell m — emitted after cell m+1's
                        sig/tanh so the ACT queue never waits on DVE."""
                        cs = cst[:, m * R:(m + 1) * R]
                        ttc = tmp.tile([128, R], bf16, tag="ttc")
                        nc.scalar.activation(ttc[:], cs, Tanh)
                        ottc[m] = ttc

                    def finishB(m, layer, t):
                        """h-mul + transpose/fp8 for cell m — trails by 2
                        cells so the DVE queue never waits on tanh(c)."""
                        tsig = osig[m]
                        ttc = ottc[m]
                        if layer == 0:
                            nc.vector.tensor_mul(
                                hs_t[:, m * HC: m * HC + R],
                                tsig[:, 2 * R:3 * R], ttc[:])
                        else:
                            hh = h1f[:, m, :]
                            nc.vector.tensor_mul(hh, tsig[:, 2 * R:3 * R],
                                                 ttc[:])
                            if t < n_steps - 1:
                                nc.vector.tensor_copy(
                                    hs_t[:, m * HC + H1OFF:
                                         m * HC + H1OFF + R], hh)
                        cols = slice(m * 128, (m + 1) * 128)
                        if layer == 0:
                            # transpose THIS m-tile's h0'|1|x|h1head cols,
                            # then its fp8 hi planes + h0' lo chunks —
                            # layer 1's m-tile m only reads its own 128
                            # columns, so it unblocks immediately
                            nc.sync.dma_start(
                                htc[:, 0:4, cols],
                                hs_t[:, m * HC: m * HC + 512],
                                transpose=True)
                            eng = nc.vector if m < 2 else nc.gpsimd
                            eng.tensor_copy(htc8[:, 0:4, cols],
                                            htc[:, 0:4, cols])
                            eng.tensor_sub(htc8[:, 7:9, cols],
                                           htc[:, 0:2, cols],
                                           htc8[:, 0:2, cols])
                        elif t < n_steps - 1:
                            # this m-tile's h1 cols (chunks 4..6); their
                            # fp8 planes are made at the top of step t+1
                            nc.sync.dma_start(
                                htc[:, 4:7, cols],
                                hs_t[:, m * HC + 512:(m + 1) * HC],
                                transpose=True)

                    for m in range(NB):
                        g = gpsum.tile([128, GN], f32, tag="g")
                        for ki, (ca, cb, _, _) in enumerate(passes):
                            st = cb - ca
                            lhsT = htc8[:, ca:cb + 1:st,
                                        m * 128:(m + 1) * 128]
                            p = poff + ki
                            for (no, nw) in NCHUNKS:
                                nc.tensor.matmul(
                                    g[:, no: no + nw],
                                    lhsT,
                                    w[:, 2 * p: 2 * p + 2, no: no + nw],
                                    start=(ki == 0),
                                    stop=(ki == nk - 1),
                                    perf_mode=DR,
                                )

                        # cell part A (torch gate order: i, f, o, g), bf16
                        cs = cst[:, m * R:(m + 1) * R]
                        tsig = tmp.tile([128, 3 * R], bf16, tag="tsig")
                        nc.scalar.activation(tsig[:], g[:, 0:3 * R], Sig)
                        tg = tmp.tile([128, R], bf16, tag="tg")
                        nc.scalar.activation(tg[:], g[:, 3 * R:4 * R], Tanh)
                        tig = tmp.tile([128, R], bf16, tag="tig")
                        nc.vector.tensor_mul(tig[:], tsig[:, 0:R], tg[:])
                        tfc = tmp.tile([128, R], bf16, tag="tfc")
                        nc.vector.tensor_mul(tfc[:], tsig[:, R:2 * R], cs)
                        nc.vector.tensor_add(cs, tfc[:], tig[:])
                        osig[m] = tsig

                        if m >= 1:
                            finishA(m - 1, cst)
                        if m >= 2:
                            finishB(m - 2, layer, t)
                    finishA(NB - 1, cst)
                    finishB(NB - 2, layer, t)
                    finishB(NB - 1, layer, t)

                # one bulk output store per step on the SP queue
                nc.sync.dma_start(
                    out_d[:, :, t * R:(t + 1) * R], h1f[:, :, :])
                if t < n_steps - 1:
                    # prefetch step t+1's weights (a full step of slack;
                    # emitted last so the Pool queue serves the mid-step
                    # fp8 converts first)
                    w_next = wpool.tile([128, NPASS * 2, GNP], f8, tag="w")
                    for c in range(3):
                        glo = c * 9
                        ghi = min((c + 1) * 9, 2 * NPASS)
                        nc.gpsimd.dma_start(
                            w_next[:, glo: ghi, :],
                            w_d[t + 1][:, glo * GNP: ghi * GNP])
                    w = w_next
    if finalize:
        nc.finalize()
    return nc


def _pack_pf(a):
    """[BC, C] -> [128, NB*C] with m-tile m at cols m*C."""
    c = a.shape[1]
    return np.ascontiguousarray(
        a.reshape(NB, 128, c).transpose(1, 0, 2).reshape(128, NB * c))


def _pack_kt(a):
    """[BC, HC] -> transposed [128, NKC*BC] with K-chunk k at cols k*BC."""
    return np.ascontiguousarray(
        a.T.reshape(NKC, 128, BC).transpose(1, 0, 2).reshape(128, NKC * BC))


def _q8(a):
    return a.astype(F8).astype(np.float32)


def prep_inputs(x, init_states_input, W_i2h0, b_i2h0, W_h2h0, b_h2h0,
                W_i2h1, b_i2h1, W_h2h1, b_h2h1, n_steps=NSTEPS):
    """Host-side packing.  Returns (in_maps, h1_init_full)."""
    x = np.asarray(x, np.float32)
    init = np.asarray(init_states_input, np.float32)
    W_i2h0 = np.asarray(W_i2h0, np.float32)
    b_i2h0 = np.asarray(b_i2h0, np.float32)
    W_h2h0 = np.asarray(W_h2h0, np.float32)
    b_h2h0 = np.asarray(b_h2h0, np.float32)
    W_i2h1 = np.asarray(W_i2h1, np.float32)
    b_i2h1 = np.asarray(b_i2h1, np.float32)
    W_h2h1 = np.asarray(W_h2h1, np.float32)
    b_h2h1 = np.asarray(b_h2h1, np.float32)

    # per-step packed-row weight planes, then per-pass fp8 blocks
    w_dev = np.zeros((n_steps, 128, NPASS * 2 * GNP), F8)
    for t in range(n_steps):
        L0row = np.zeros((512, GN), np.float32)
        L0row[0:R] = W_h2h0[t].T
        L0row[R] = b_i2h0[t] + b_h2h0[t]
        L0row[R + 1: R + 1 + IN] = W_i2h0[t].T
        L1row = np.zeros((HC, GN), np.float32)
        L1row[0:R] = W_i2h1[t].T
        L1row[R] = b_i2h1[t] + b_h2h1[t]
        L1row[H1OFF: H1OFF + R] = W_h2h1[t].T

        L0hi = _q8(L0row)
        L0lo = _q8(L0row - L0hi)
        L1hi = _q8(L1row)
        L1lo = _q8(L1row - L1hi)
        planes = {"h0": L0hi, "l0": L0lo, "h1": L1hi, "l1": L1lo}

        Wt = np.zeros((NPASS, 2, 128, GNP), np.float32)
        for p, (_, _, wa, wb) in enumerate(L0P + L1P):
            for gsel, (plane, ck) in ((0, wa), (1, wb)):
                Wt[p, gsel, :, 0:GN] = planes[plane][ck * 128:(ck + 1) * 128]
        # -> [128, NPASS*2*GNP]
        w_dev[t] = np.ascontiguousarray(
            Wt.transpose(2, 0, 1, 3).reshape(128, NPASS * 2 * GNP)).astype(F8)

    init4 = init.reshape(B, 4, R)
    h0_full, c0_full = init4[:, 0], init4[:, 1]
    h1_full, c1_full = init4[:, 2], init4[:, 3]

    in_maps = []
    for c in range(NCORES):
        sl = slice(c * BC, (c + 1) * BC)
        hcp = np.zeros((BC, HC), np.float32)
        hcp[:, 0:R] = h0_full[sl]
        hcp[:, R] = 1.0
        hcp[:, R + 1: R + 1 + IN] = x[sl]
        hcp[:, H1OFF: H1OFF + R] = h1_full[sl]
        hcp = hcp.astype(BF16)
        in_maps.append({
            "w": w_dev,
            "hci": _pack_pf(hcp),
            "htci": _pack_kt(hcp),
            "c0i": _pack_pf(np.ascontiguousarray(c0_full[sl])).astype(BF16),
            "c1i": _pack_pf(np.ascontiguousarray(c1_full[sl])).astype(BF16),
        })
    return in_maps, h1_full


def kernel(x, init_states_input, W_i2h0, b_i2h0, W_h2h0, b_h2h0,
           W_i2h1, b_i2h1, W_h2h1, b_h2h1):
    global LAST_RESULT
    from concourse.bass_utils import run_bass_kernel_spmd

    in_maps, h1_full = prep_inputs(
        x, init_states_input, W_i2h0, b_i2h0, W_h2h0, b_h2h0,
        W_i2h1, b_i2h1, W_h2h1, b_h2h1)

    nc = build_bass(NSTEPS)
    res = run_bass_kernel_spmd(nc, in_maps, list(range(NCORES)), trace=TRACE)
    LAST_RESULT = res

    out = np.empty((B, (NSTEPS + 1) * R), np.float32)
    out[:, 0:R] = h1_full
    for c in range(NCORES):
        # device out[p, m, :] = batch row m*128+p
        dev = res.results[c]["out"]
        out[c * BC:(c + 1) * BC, R:] = (
            dev.transpose(1, 0, 2).reshape(BC, NSTEPS * R))
    return out


# revision 43
# speedup vs baseline: 1.3858x; 1.1341x over previous
"""Bass/Trainium2 kernel for nn_BuildLstmUnrollNet — fp8 DoubleRow version.

Problem: 2-layer LSTM, unrolled T=11 steps with per-step (non-shared)
weights, B=8192, R=425, IN=20.  Output block t is the last-layer h
*before* step t, so only steps 0..9 need computing.

Strategy (data-parallel over batch, 8 cores x 1024 rows):
  - Gates are computed batch-major with the *transposed activations*
    stationary (lhsT) in fp8e4 and the weights moving in fp8e4, using
    perf_mode=DoubleRow (2 fp8 MACs/cell/cycle -> 2x PE throughput).
  - Error compensation to stay inside the 2e-2 gate:
      * weights split hi+lo (two fp8 planes, effective ~11-bit weights)
      * activations: fp8-hi everywhere plus fp8-lo "compensation rows"
        for 256 of the 425 h rows per tensor (h0, h0', h1), carried as
        extra K-chunks whose weight blocks are the hi planes.
    Per m-tile per step: layer0 = 5 DoubleRow passes, layer1 = 9.
  - The recurrent transpose h -> hT runs directly SBUF->SBUF on the
    xbar (bf16), no DRAM bounce; DVE converts the transposed bf16 to
    the fp8 hi plane and computes the fp8 lo plane (hi-lo subtract).
  - Cell math in bf16 on DVE (2x mode), c kept bf16; one fused sigmoid
    over i|f|o + tanh(g) straight out of PSUM on ACT; tanh(c) merged
    over m-tile pairs.

kernel(**inputs) takes full-size numpy inputs, does the host-side
packing/sharding, runs the same program SPMD on cores 0..7, and
reassembles the full [8192, 4675] fp32 output (block 0 comes straight
from the initial state on the host).
"""

import numpy as np
import ml_dtypes

BF16 = ml_dtypes.bfloat16
F8 = ml_dtypes.float8_e4m3fn

B = 8192
NCORES = 8
BC = B // NCORES          # batch rows per core (1024)
NB = BC // 128            # m-tiles per core (8)
R = 425
IN = 20
GN = 4 * R                # 1700 gate columns
GNP = 1712                # 16-aligned weight group stride
H1OFF = R + 1 + IN        # 446: h1 col offset in the packed state block
HC = 896                  # packed state block width (7*128)
NKC = HC // 128           # 7 hi chunks
NCH = 11                  # 7 hi + 4 lo chunks (7=lo c0, 8=lo c1, 9=lo c4, 10=lo c5)
NSTEPS = 10
NCHUNKS_A = [(0, 512), (512, 512), (1024, 251)]   # i|f|o -> tile A
NCHUNKS_B = [(1275, 425)]                         # g gate -> tile B

# Pass plans: (chunkA, chunkB, wsrcA, wsrcB); wsrc = (plane, chunk) with
# plane in {h0,l0} (layer-0 hi/lo) or {h1,l1} (layer-1 hi/lo).
L0P = [
    (0, 1, ("h0", 0), ("h0", 1)),
    (2, 3, ("h0", 2), ("h0", 3)),
    (0, 1, ("l0", 0), ("l0", 1)),
    (2, 3, ("l0", 2), ("l0", 3)),
]
# h0'-only passes first, h1-dependent chunks (3 is mid-step, 4..6,9,10 are
# end-of-previous-step) last, so layer-1 matmuls start before the h1
# fp8 planes for this step are finished
L1P = [
    (0, 1, ("h1", 0), ("h1", 1)),
    (2, 3, ("h1", 2), ("h1", 3)),
    (0, 1, ("l1", 0), ("l1", 1)),
    (2, 3, ("l1", 2), ("l1", 3)),
    (6, 7, ("h1", 6), ("h1", 0)),      # + act-comp h0' rows 0..127
    (6, 8, ("l1", 6), ("h1", 1)),      # + act-comp h0' rows 128..255
    (4, 5, ("h1", 4), ("h1", 5)),
    (4, 5, ("l1", 4), ("l1", 5)),
    (9, 10, ("h1", 4), ("h1", 5)),     # act-comp h1 rows 66..321
]
NPASS = len(L0P) + len(L1P)            # 14

# set by test.py to profile; results stashed in LAST_RESULT
TRACE = False
LAST_RESULT = None


def build_bass(n_steps=NSTEPS, finalize=True):
    import concourse.bacc as bacc
    import concourse.mybir as mybir
    import concourse.tile as tile

    f32 = mybir.dt.float32
    bf16 = mybir.dt.bfloat16
    f8 = mybir.dt.float8e4
    DR = mybir.MatmulPerfMode.DoubleRow
    Sig = mybir.ActivationFunctionType.Sigmoid
    Tanh = mybir.ActivationFunctionType.Tanh

    nc = bacc.Bacc()

    w_d = nc.declare_dram_parameter("w", [n_steps, 128, NPASS * 2 * GNP], f8,
                                    False)
    hci_d = nc.declare_dram_parameter("hci", [128, NB * HC], bf16, False)
    htci_d = nc.declare_dram_parameter("htci", [128, NKC * BC], bf16, False)
    c0i_d = nc.declare_dram_parameter("c0i", [128, NB * R], bf16, False)
    c1i_d = nc.declare_dram_parameter("c1i", [128, NB * R], bf16, False)
    # out[p, m, t*R+r] = h1 block t for batch row m*128+p (host reassembles)
    out_d = nc.declare_dram_parameter("out", [128, NB, n_steps * R], bf16, True)

    with tile.TileContext(nc) as tc:
        with (
            tc.tile_pool(name="consts", bufs=1) as consts,
            tc.tile_pool(name="wpool", bufs=2) as wpool,
            tc.tile_pool(name="gpsum", bufs=2, space="PSUM") as gpsum,
            tc.tile_pool(name="tmp", bufs=6) as tmp,
            tc.tile_pool(name="h1pool", bufs=2) as h1pool,
        ):
            # persistent state tiles
            hs_t = consts.tile([128, NB * HC], bf16)   # packed batch-major
            htc = consts.tile([128, NKC, BC], bf16)    # transposed, bf16
            htc8 = consts.tile([128, NCH, BC], f8)     # fp8 hi + lo chunks
            c0 = consts.tile([128, NB * R], bf16)
            c1 = consts.tile([128, NB * R], bf16)

            # init DMAs on the SP (HWDGE) queue while the first weight
            # chunks stream on the Pool (SWDGE) queue
            for k in range(NKC):
                nc.sync.dma_start(htc[:, k, :], htci_d[:, k * BC:(k + 1) * BC])
            nc.sync.dma_start(c0[:], c0i_d[:])
            nc.sync.dma_start(hs_t[:], hci_d[:])
            nc.sync.dma_start(c1[:], c1i_d[:])

            # step-0 weights, split per pass-pair so matmuls start early
            w = wpool.tile([128, NPASS * 2, GNP], f8, tag="w")
            for p in range(NPASS):
                nc.gpsimd.dma_start(
                    w[:, 2 * p: 2 * p + 2, :],
                    w_d[0][:, 2 * p * GNP: (2 * p + 2) * GNP])

            # initial fp8 conversion of the transposed state
            nc.vector.tensor_copy(htc8[:, 0:NKC, :], htc[:, 0:NKC, :])
            nc.vector.tensor_sub(htc8[:, 7:9, :], htc[:, 0:2, :],
                                 htc8[:, 0:2, :])
            nc.vector.tensor_sub(htc8[:, 9:11, :], htc[:, 4:6, :],
                                 htc8[:, 4:6, :])

            # PE warm-up: the HAM clock gate needs ~3.4us of sustained
            # activity before the PE runs at full rate.
            warm = consts.tile([128, 128], bf16)
            nc.vector.memset(warm[:], 0.0)
            wps = gpsum.tile([128, 3 * R], f32, tag="ga")
            for i in range(20):
                nc.tensor.matmul(wps[:, 0:128], warm[:], warm[:],
                                 start=True, stop=True)



            for t in range(n_steps):
                h1f = h1pool.tile([128, NB, R], bf16, tag="h1f")
                if t > 0:
                    # h1 chunks (4..6) were transposed at the end of step
                    # t-1; produce their fp8 hi planes + h1 lo chunks on
                    # GPSIMD so neither the DVE nor the ACT in-order
                    # queues ever wait on them.  Layer 1's h1 passes (the
                    # last 5 of L1P) are the only consumers.  These go
                    # FIRST on the Pool queue: the w prefetch below has a
                    # WAR wait on step t-1's matmuls and a step of slack.
                    nc.gpsimd.tensor_copy(htc8[:, 4:7, :], htc[:, 4:7, :])
                    nc.gpsimd.tensor_sub(htc8[:, 9:11, :], htc[:, 4:6, :],
                                         htc8[:, 4:6, :])

                for layer in range(2):
                    passes = L0P if layer == 0 else L1P
                    poff = 0 if layer == 0 else len(L0P)
                    nk = len(passes)
                    cst = c0 if layer == 0 else c1
                    osig = [None] * NB   # per-m tsig kept for finishB()
                    ottc = [None] * NB   # per-m tanh(c) kept for finishB()

                    def finishA(m, cst):
                        """tanh(c) for cell m — emitted after cell m+1's
                        sig/tanh so the ACT queue never waits on DVE."""
                        cs = cst[:, m * R:(m + 1) * R]
                        ttc = tmp.tile([128, R], bf16, tag="ttc")
                        nc.scalar.activation(ttc[:], cs, Tanh)
                        ottc[m] = ttc

                    def finishB(m, layer, t):
                        """h-mul + transpose/fp8 for cell m — trails by 2
                        cells so the DVE queue never waits on tanh(c)."""
                        tsig = osig[m]
                        ttc = ottc[m]
                        if layer == 0:
                            nc.vector.tensor_mul(
                                hs_t[:, m * HC: m * HC + R],
                                tsig[:, 2 * R:3 * R], ttc[:])
                        else:
                            hh = h1f[:, m, :]
                            nc.vector.tensor_mul(hh, tsig[:, 2 * R:3 * R],
                                                 ttc[:])
                            if t < n_steps - 1:
                                nc.vector.tensor_copy(
                                    hs_t[:, m * HC + H1OFF:
                                         m * HC + H1OFF + R], hh)
                        cols = slice(m * 128, (m + 1) * 128)
                        if layer == 0:
                            # transpose THIS m-tile's h0'|1|x|h1head cols,
                            # then its fp8 hi planes + h0' lo chunks —
                            # layer 1's m-tile m only reads its own 128
                            # columns, so it unblocks immediately
                            nc.sync.dma_start(
                                htc[:, 0:4, cols],
                                hs_t[:, m * HC: m * HC + 512],
                                transpose=True)
                            eng = nc.vector if m < 2 else nc.gpsimd
                            eng.tensor_copy(htc8[:, 0:4, cols],
                                            htc[:, 0:4, cols])
                            eng.tensor_sub(htc8[:, 7:9, cols],
                                           htc[:, 0:2, cols],
                                           htc8[:, 0:2, cols])
                        elif t < n_steps - 1:
                            # this m-tile's h1 cols (chunks 4..6); their
                            # fp8 planes are made at the top of step t+1
                            nc.sync.dma_start(
                                htc[:, 4:7, cols],
                                hs_t[:, m * HC + 512:(m + 1) * HC],
                                transpose=True)

                    for m in range(NB):
                        g_a = gpsum.tile([128, 3 * R], f32, tag="ga")
                        g_b = gpsum.tile([128, R], f32, tag="gb")
                        for ki, (ca, cb, _, _) in enumerate(passes):
                            st = cb - ca
                            lhsT = htc8[:, ca:cb + 1:st,
                                        m * 128:(m + 1) * 128]
                            p = poff + ki
                            for (no, nw) in NCHUNKS_A:
                                nc.tensor.matmul(
                                    g_a[:, no: no + nw],
                                    lhsT,
                                    w[:, 2 * p: 2 * p + 2, no: no + nw],
                                    start=(ki == 0),
                                    stop=(ki == nk - 1),
                                    perf_mode=DR,
                                )
                            for (no, nw) in NCHUNKS_B:
                                nc.tensor.matmul(
                                    g_b[:, no - 3 * R: no - 3 * R + nw],
                                    lhsT,
                                    w[:, 2 * p: 2 * p + 2, no: no + nw],
                                    start=(ki == 0),
                                    stop=(ki == nk - 1),
                                    perf_mode=DR,
                                )

                        # cell part A (torch gate order: i, f, o, g), bf16
                        cs = cst[:, m * R:(m + 1) * R]
                        tsig = tmp.tile([128, 3 * R], bf16, tag="tsig")
                        nc.scalar.activation(tsig[:], g_a[:, 0:3 * R], Sig)
                        tg = tmp.tile([128, R], bf16, tag="tg")
                        nc.scalar.activation(tg[:], g_b[:, 0:R], Tanh)
                        tig = tmp.tile([128, R], bf16, tag="tig")
                        nc.vector.tensor_mul(tig[:], tsig[:, 0:R], tg[:])
                        tfc = tmp.tile([128, R], bf16, tag="tfc")
                        nc.vector.tensor_mul(tfc[:], tsig[:, R:2 * R], cs)
                        nc.vector.tensor_add(cs, tfc[:], tig[:])
                        osig[m] = tsig

                        if m >= 1:
                            finishA(m - 1, cst)
                        if m >= 2:
                            finishB(m - 2, layer, t)
                    finishA(NB - 1, cst)
                    finishB(NB - 2, layer, t)
                    finishB(NB - 1, layer, t)

                # output store per step on the SP queue, two pieces
                nc.sync.dma_start(
                    out_d[:, 0:4, t * R:(t + 1) * R], h1f[:, 0:4, :])
                nc.sync.dma_start(
                    out_d[:, 4:8, t * R:(t + 1) * R], h1f[:, 4:8, :])
                if t < n_steps - 1:
                    # prefetch step t+1's weights (a full step of slack;
                    # emitted last so the Pool queue serves the mid-step
                    # fp8 converts first)
                    w_next = wpool.tile([128, NPASS * 2, GNP], f8, tag="w")
                    for c in range(4):
                        glo = c * 7
                        ghi = min((c + 1) * 7, 2 * NPASS)
                        nc.gpsimd.dma_start(
                            w_next[:, glo: ghi, :],
                            w_d[t + 1][:, glo * GNP: ghi * GNP])
                    w = w_next
    if finalize:
        nc.finalize()
    return nc


def _pack_pf(a):
    """[BC, C] -> [128, NB*C] with m-tile m at cols m*C."""
    c = a.shape[1]
    return np.ascontiguousarray(
        a.reshape(NB, 128, c).transpose(1, 0, 2).reshape(128, NB * c))


def _pack_kt(a):
    """[BC, HC] -> transposed [128, NKC*BC] with K-chunk k at cols k*BC."""
    return np.ascontiguousarray(
        a.T.reshape(NKC, 128, BC).transpose(1, 0, 2).reshape(128, NKC * BC))


def _q8(a):
    return a.astype(F8).astype(np.float32)


def prep_inputs(x, init_states_input, W_i2h0, b_i2h0, W_h2h0, b_h2h0,
                W_i2h1, b_i2h1, W_h2h1, b_h2h1, n_steps=NSTEPS):
    """Host-side packing.  Returns (in_maps, h1_init_full)."""
    x = np.asarray(x, np.float32)
    init = np.asarray(init_states_input, np.float32)
    W_i2h0 = np.asarray(W_i2h0, np.float32)
    b_i2h0 = np.asarray(b_i2h0, np.float32)
    W_h2h0 = np.asarray(W_h2h0, np.float32)
    b_h2h0 = np.asarray(b_h2h0, np.float32)
    W_i2h1 = np.asarray(W_i2h1, np.float32)
    b_i2h1 = np.asarray(b_i2h1, np.float32)
    W_h2h1 = np.asarray(W_h2h1, np.float32)
    b_h2h1 = np.asarray(b_h2h1, np.float32)

    # per-step packed-row weight planes, then per-pass fp8 blocks
    w_dev = np.zeros((n_steps, 128, NPASS * 2 * GNP), F8)
    for t in range(n_steps):
        L0row = np.zeros((512, GN), np.float32)
        L0row[0:R] = W_h2h0[t].T
        L0row[R] = b_i2h0[t] + b_h2h0[t]
        L0row[R + 1: R + 1 + IN] = W_i2h0[t].T
        L1row = np.zeros((HC, GN), np.float32)
        L1row[0:R] = W_i2h1[t].T
        L1row[R] = b_i2h1[t] + b_h2h1[t]
        L1row[H1OFF: H1OFF + R] = W_h2h1[t].T

        L0hi = _q8(L0row)
        L0lo = _q8(L0row - L0hi)
        L1hi = _q8(L1row)
        L1lo = _q8(L1row - L1hi)
        planes = {"h0": L0hi, "l0": L0lo, "h1": L1hi, "l1": L1lo}

        Wt = np.zeros((NPASS, 2, 128, GNP), np.float32)
        for p, (_, _, wa, wb) in enumerate(L0P + L1P):
            for gsel, (plane, ck) in ((0, wa), (1, wb)):
                Wt[p, gsel, :, 0:GN] = planes[plane][ck * 128:(ck + 1) * 128]
        # -> [128, NPASS*2*GNP]
        w_dev[t] = np.ascontiguousarray(
            Wt.transpose(2, 0, 1, 3).reshape(128, NPASS * 2 * GNP)).astype(F8)

    init4 = init.reshape(B, 4, R)
    h0_full, c0_full = init4[:, 0], init4[:, 1]
    h1_full, c1_full = init4[:, 2], init4[:, 3]

    in_maps = []
    for c in range(NCORES):
        sl = slice(c * BC, (c + 1) * BC)
        hcp = np.zeros((BC, HC), np.float32)
        hcp[:, 0:R] = h0_full[sl]
        hcp[:, R] = 1.0
        hcp[:, R + 1: R + 1 + IN] = x[sl]
        hcp[:, H1OFF: H1OFF + R] = h1_full[sl]
        hcp = hcp.astype(BF16)
        in_maps.append({
            "w": w_dev,
            "hci": _pack_pf(hcp),
            "htci": _pack_kt(hcp),
            "c0i": _pack_pf(np.ascontiguousarray(c0_full[sl])).astype(BF16),
            "c1i": _pack_pf(np.ascontiguousarray(c1_full[sl])).astype(BF16),
        })
    return in_maps, h1_full


def kernel(x, init_states_input, W_i2h0, b_i2h0, W_h2h0, b_h2h0,
           W_i2h1, b_i2h1, W_h2h1, b_h2h1):
    global LAST_RESULT
    from concourse.bass_utils import run_bass_kernel_spmd

    in_maps, h1_full = prep_inputs(
        x, init_states_input, W_i2h0, b_i2h0, W_h2h0, b_h2h0,
        W_i2h1, b_i2h1, W_h2h1, b_h2h1)

    nc = build_bass(NSTEPS)
    res = run_bass_kernel_spmd(nc, in_maps, list(range(NCORES)), trace=TRACE)
    LAST_RESULT = res

    out = np.empty((B, (NSTEPS + 1) * R), np.float32)
    out[:, 0:R] = h1_full
    for c in range(NCORES):
        # device out[p, m, :] = batch row m*128+p
        dev = np.asarray(res.results[c]["out"], np.float32)
        out[c * BC:(c + 1) * BC, R:] = (
            dev.transpose(1, 0, 2).reshape(BC, NSTEPS * R))
    return out


# revision 49
# speedup vs baseline: 1.4084x; 1.0163x over previous
"""Bass/Trainium2 kernel for nn_BuildLstmUnrollNet — fp8 DoubleRow version.

Problem: 2-layer LSTM, unrolled T=11 steps with per-step (non-shared)
weights, B=8192, R=425, IN=20.  Output block t is the last-layer h
*before* step t, so only steps 0..9 need computing.

Strategy (data-parallel over batch, 8 cores x 1024 rows):
  - Gates are computed batch-major with the *transposed activations*
    stationary (lhsT) in fp8e4 and the weights moving in fp8e4, using
    perf_mode=DoubleRow (2 fp8 MACs/cell/cycle -> 2x PE throughput).
  - Error compensation to stay inside the 2e-2 gate:
      * weights split hi+lo (two fp8 planes, effective ~11-bit weights)
      * activations: fp8-hi everywhere plus fp8-lo "compensation rows"
        for 256 of the 425 h rows per tensor (h0, h0', h1), carried as
        extra K-chunks whose weight blocks are the hi planes.
    Per m-tile per step: layer0 = 5 DoubleRow passes, layer1 = 9.
  - The recurrent transpose h -> hT runs directly SBUF->SBUF on the
    xbar (bf16), no DRAM bounce; DVE converts the transposed bf16 to
    the fp8 hi plane and computes the fp8 lo plane (hi-lo subtract).
  - Cell math in bf16 on DVE (2x mode), c kept bf16; one fused sigmoid
    over i|f|o + tanh(g) straight out of PSUM on ACT; tanh(c) merged
    over m-tile pairs.

kernel(**inputs) takes full-size numpy inputs, does the host-side
packing/sharding, runs the same program SPMD on cores 0..7, and
reassembles the full [8192, 4675] fp32 output (block 0 comes straight
from the initial state on the host).
"""

import numpy as np
import ml_dtypes

BF16 = ml_dtypes.bfloat16
F8 = ml_dtypes.float8_e4m3fn

B = 8192
NCORES = 8
BC = B // NCORES          # batch rows per core (1024)
NB = BC // 128            # m-tiles per core (8)
R = 425
IN = 20
GN = 4 * R                # 1700 gate columns
GNP = 1712                # 16-aligned weight group stride
H1OFF = R + 1 + IN        # 446: h1 col offset in the packed state block
HC = 896                  # packed state block width (7*128)
NKC = HC // 128           # 7 hi chunks
NCH = 11                  # 7 hi + 4 lo chunks (7=lo c0, 8=lo c1, 9=lo c4, 10=lo c5)
NSTEPS = 10
NCHUNKS_A = [(0, 512), (512, 512), (1024, 251)]   # i|f|o -> tile A
NCHUNKS_B = [(1275, 425)]                         # g gate -> tile B

# Pass plans: (chunkA, chunkB, wsrcA, wsrcB); wsrc = (plane, chunk) with
# plane in {h0,l0} (layer-0 hi/lo) or {h1,l1} (layer-1 hi/lo).
L0P = [
    (0, 1, ("h0", 0), ("h0", 1)),
    (2, 3, ("h0", 2), ("h0", 3)),
    (0, 1, ("l0", 0), ("l0", 1)),
    (2, 3, ("l0", 2), ("l0", 3)),
]
# h0'-only passes first, h1-dependent chunks (3 is mid-step, 4..6,9,10 are
# end-of-previous-step) last, so layer-1 matmuls start before the h1
# fp8 planes for this step are finished
L1P = [
    (0, 1, ("h1", 0), ("h1", 1)),
    (2, 3, ("h1", 2), ("h1", 3)),
    (0, 1, ("l1", 0), ("l1", 1)),
    (2, 3, ("l1", 2), ("l1", 3)),
    (6, 7, ("h1", 6), ("h1", 0)),      # + act-comp h0' rows 0..127
    (6, 8, ("l1", 6), ("h1", 1)),      # + act-comp h0' rows 128..255
    (4, 5, ("h1", 4), ("h1", 5)),
    (4, 5, ("l1", 4), ("l1", 5)),
    (9, 10, ("h1", 4), ("h1", 5)),     # act-comp h1 rows 66..321
]
NPASS = len(L0P) + len(L1P)            # 14

# set by test.py to profile; results stashed in LAST_RESULT
TRACE = False
LAST_RESULT = None


def build_bass(n_steps=NSTEPS, finalize=True):
    import concourse.bacc as bacc
    import concourse.mybir as mybir
    import concourse.tile as tile

    f32 = mybir.dt.float32
    bf16 = mybir.dt.bfloat16
    f8 = mybir.dt.float8e4
    DR = mybir.MatmulPerfMode.DoubleRow
    Sig = mybir.ActivationFunctionType.Sigmoid
    Tanh = mybir.ActivationFunctionType.Tanh

    nc = bacc.Bacc()

    w_d = nc.declare_dram_parameter("w", [n_steps, 128, NPASS * 2 * GNP], f8,
                                    False)
    hci_d = nc.declare_dram_parameter("hci", [128, NB * HC], bf16, False)
    htci_d = nc.declare_dram_parameter("htci", [128, NKC * BC], bf16, False)
    c0i_d = nc.declare_dram_parameter("c0i", [128, NB * R], bf16, False)
    c1i_d = nc.declare_dram_parameter("c1i", [128, NB * R], bf16, False)
    # out[p, m, t*R+r] = h1 block t for batch row m*128+p (host reassembles)
    out_d = nc.declare_dram_parameter("out", [128, NB, n_steps * R], bf16, True)

    with tile.TileContext(nc) as tc:
        with (
            tc.tile_pool(name="consts", bufs=1) as consts,
            tc.tile_pool(name="wpool", bufs=2) as wpool,
            tc.tile_pool(name="gpsum", bufs=2, space="PSUM") as gpsum,
            tc.tile_pool(name="tmp", bufs=6) as tmp,
            tc.tile_pool(name="h1pool", bufs=2) as h1pool,
        ):
            # persistent state tiles
            hs_t = consts.tile([128, NB * HC], bf16)   # packed batch-major
            htc = consts.tile([128, NKC, BC], bf16)    # transposed, bf16
            htc8 = consts.tile([128, NCH, BC], f8)     # fp8 hi + lo chunks
            c0 = consts.tile([128, NB * R], bf16)
            c1 = consts.tile([128, NB * R], bf16)

            # init DMAs on the SP (HWDGE) queue while the first weight
            # chunks stream on the Pool (SWDGE) queue
            for k in range(NKC):
                nc.sync.dma_start(htc[:, k, :], htci_d[:, k * BC:(k + 1) * BC])
            nc.sync.dma_start(c0[:], c0i_d[:])
            nc.sync.dma_start(hs_t[:], hci_d[:])
            nc.sync.dma_start(c1[:], c1i_d[:])

            # step-0 weights, split per pass-pair so matmuls start early
            w = wpool.tile([128, NPASS * 2, GNP], f8, tag="w")
            for p in range(NPASS):
                nc.gpsimd.dma_start(
                    w[:, 2 * p: 2 * p + 2, :],
                    w_d[0][:, 2 * p * GNP: (2 * p + 2) * GNP])

            # initial fp8 conversion of the transposed state
            nc.vector.tensor_copy(htc8[:, 0:NKC, :], htc[:, 0:NKC, :])
            nc.vector.tensor_sub(htc8[:, 7:9, :], htc[:, 0:2, :],
                                 htc8[:, 0:2, :])
            nc.vector.tensor_sub(htc8[:, 9:11, :], htc[:, 4:6, :],
                                 htc8[:, 4:6, :])

            # PE warm-up: the HAM clock gate needs ~3.4us of sustained
            # activity before the PE runs at full rate.
            warm = consts.tile([128, 128], bf16)
            nc.vector.memset(warm[:], 0.0)
            wps = gpsum.tile([128, 3 * R], f32, tag="ga")
            for i in range(20):
                nc.tensor.matmul(wps[:, 0:128], warm[:], warm[:],
                                 start=True, stop=True)



            for t in range(n_steps):
                h1f = h1pool.tile([128, NB, R], bf16, tag="h1f")
                if t > 0:
                    # h1 chunks (4..6) were transposed at the end of step
                    # t-1; produce their fp8 hi planes + h1 lo chunks on
                    # GPSIMD so neither the DVE nor the ACT in-order
                    # queues ever wait on them.  Layer 1's h1 passes (the
                    # last 5 of L1P) are the only consumers.  These go
                    # FIRST on the Pool queue: the w prefetch below has a
                    # WAR wait on step t-1's matmuls and a step of slack.
                    nc.gpsimd.tensor_copy(htc8[:, 4:7, :], htc[:, 4:7, :])
                    nc.gpsimd.tensor_sub(htc8[:, 9:11, :], htc[:, 4:6, :],
                                         htc8[:, 4:6, :])

                for layer in range(2):
                    passes = L0P if layer == 0 else L1P
                    poff = 0 if layer == 0 else len(L0P)
                    nk = len(passes)
                    cst = c0 if layer == 0 else c1
                    osig = [None] * NB   # per-m tsig kept for finishB()
                    ottc = [None] * NB   # per-m tanh(c) kept for finishB()

                    def finishA(m, cst):
                        """tanh(c), merged over m-tile pairs (c is
                        contiguous across m) — emitted after cell m+1's
                        sig/tanh so the ACT queue never waits on DVE."""
                        if m % 2 == 0:
                            return          # emitted with its pair partner
                        cs = cst[:, (m - 1) * R:(m + 1) * R]
                        ttc = tmp.tile([128, 2 * R], bf16, tag="ttc")
                        nc.scalar.activation(ttc[:], cs, Tanh)
                        ottc[m - 1] = ttc[:, 0:R]
                        ottc[m] = ttc[:, R:2 * R]

                    def finishB(m, layer, t):
                        """h-mul + transpose/fp8 for cell m — trails by 2
                        cells so the DVE queue never waits on tanh(c)."""
                        tsig = osig[m]
                        ttc = ottc[m]
                        if layer == 0:
                            nc.vector.tensor_mul(
                                hs_t[:, m * HC: m * HC + R],
                                tsig[:, 2 * R:3 * R], ttc[:])
                        else:
                            hh = h1f[:, m, :]
                            nc.vector.tensor_mul(hh, tsig[:, 2 * R:3 * R],
                                                 ttc[:])
                            if t < n_steps - 1:
                                nc.vector.tensor_copy(
                                    hs_t[:, m * HC + H1OFF:
                                         m * HC + H1OFF + R], hh)
                        cols = slice(m * 128, (m + 1) * 128)
                        if layer == 0:
                            # transpose THIS m-tile's h0'|1|x|h1head cols,
                            # then its fp8 hi planes + h0' lo chunks —
                            # layer 1's m-tile m only reads its own 128
                            # columns, so it unblocks immediately
                            nc.sync.dma_start(
                                htc[:, 0:4, cols],
                                hs_t[:, m * HC: m * HC + 512],
                                transpose=True)
                            eng = nc.vector if m < 2 else nc.gpsimd
                            eng.tensor_copy(htc8[:, 0:4, cols],
                                            htc[:, 0:4, cols])
                            eng.tensor_sub(htc8[:, 7:9, cols],
                                           htc[:, 0:2, cols],
                                           htc8[:, 0:2, cols])
                        elif t < n_steps - 1:
                            # this m-tile's h1 cols (chunks 4..6); their
                            # fp8 planes are made at the top of step t+1
                            nc.sync.dma_start(
                                htc[:, 4:7, cols],
                                hs_t[:, m * HC + 512:(m + 1) * HC],
                                transpose=True)

                    for m in range(NB):
                        g_a = gpsum.tile([128, 3 * R], f32, tag="ga")
                        g_b = gpsum.tile([128, R], f32, tag="gb")
                        for ki, (ca, cb, _, _) in enumerate(passes):
                            st = cb - ca
                            lhsT = htc8[:, ca:cb + 1:st,
                                        m * 128:(m + 1) * 128]
                            p = poff + ki
                            for (no, nw) in NCHUNKS_A:
                                nc.tensor.matmul(
                                    g_a[:, no: no + nw],
                                    lhsT,
                                    w[:, 2 * p: 2 * p + 2, no: no + nw],
                                    start=(ki == 0),
                                    stop=(ki == nk - 1),
                                    perf_mode=DR,
                                )
                            for (no, nw) in NCHUNKS_B:
                                nc.tensor.matmul(
                                    g_b[:, no - 3 * R: no - 3 * R + nw],
                                    lhsT,
                                    w[:, 2 * p: 2 * p + 2, no: no + nw],
                                    start=(ki == 0),
                                    stop=(ki == nk - 1),
                                    perf_mode=DR,
                                )

                        # cell part A (torch gate order: i, f, o, g), bf16
                        cs = cst[:, m * R:(m + 1) * R]
                        tsig = tmp.tile([128, 3 * R], bf16, tag="tsig")
                        nc.scalar.activation(tsig[:], g_a[:, 0:3 * R], Sig)
                        tg = tmp.tile([128, R], bf16, tag="tg")
                        nc.scalar.activation(tg[:], g_b[:, 0:R], Tanh)
                        tig = tmp.tile([128, R], bf16, tag="tig")
                        nc.vector.tensor_mul(tig[:], tsig[:, 0:R], tg[:])
                        tfc = tmp.tile([128, R], bf16, tag="tfc")
                        nc.vector.tensor_mul(tfc[:], tsig[:, R:2 * R], cs)
                        nc.vector.tensor_add(cs, tfc[:], tig[:])
                        osig[m] = tsig

                        if m >= 1:
                            finishA(m - 1, cst)
                        if m >= 2:
                            finishB(m - 2, layer, t)
                    finishA(NB - 1, cst)
                    finishB(NB - 2, layer, t)
                    finishB(NB - 1, layer, t)

                # output store per step on the SP queue, two pieces
                nc.sync.dma_start(
                    out_d[:, 0:4, t * R:(t + 1) * R], h1f[:, 0:4, :])
                nc.sync.dma_start(
                    out_d[:, 4:8, t * R:(t + 1) * R], h1f[:, 4:8, :])
                if t < n_steps - 1:
                    # prefetch step t+1's weights (a full step of slack;
                    # emitted last so the Pool queue serves the mid-step
                    # fp8 converts first)
                    w_next = wpool.tile([128, NPASS * 2, GNP], f8, tag="w")
                    for c in range(4):
                        glo = c * 7
                        ghi = min((c + 1) * 7, 2 * NPASS)
                        nc.gpsimd.dma_start(
                            w_next[:, glo: ghi, :],
                            w_d[t + 1][:, glo * GNP: ghi * GNP])
                    w = w_next
    if finalize:
        nc.finalize()
    return nc


def _pack_pf(a):
    """[BC, C] -> [128, NB*C] with m-tile m at cols m*C."""
    c = a.shape[1]
    return np.ascontiguousarray(
        a.reshape(NB, 128, c).transpose(1, 0, 2).reshape(128, NB * c))


def _pack_kt(a):
    """[BC, HC] -> transposed [128, NKC*BC] with K-chunk k at cols k*BC."""
    return np.ascontiguousarray(
        a.T.reshape(NKC, 128, BC).transpose(1, 0, 2).reshape(128, NKC * BC))


def _q8(a):
    return a.astype(F8).astype(np.float32)


def prep_inputs(x, init_states_input, W_i2h0, b_i2h0, W_h2h0, b_h2h0,
                W_i2h1, b_i2h1, W_h2h1, b_h2h1, n_steps=NSTEPS):
    """Host-side packing.  Returns (in_maps, h1_init_full)."""
    x = np.asarray(x, np.float32)
    init = np.asarray(init_states_input, np.float32)
    W_i2h0 = np.asarray(W_i2h0, np.float32)
    b_i2h0 = np.asarray(b_i2h0, np.float32)
    W_h2h0 = np.asarray(W_h2h0, np.float32)
    b_h2h0 = np.asarray(b_h2h0, np.float32)
    W_i2h1 = np.asarray(W_i2h1, np.float32)
    b_i2h1 = np.asarray(b_i2h1, np.float32)
    W_h2h1 = np.asarray(W_h2h1, np.float32)
    b_h2h1 = np.asarray(b_h2h1, np.float32)

    # per-step packed-row weight planes, then per-pass fp8 blocks
    w_dev = np.zeros((n_steps, 128, NPASS * 2 * GNP), F8)
    for t in range(n_steps):
        L0row = np.zeros((512, GN), np.float32)
        L0row[0:R] = W_h2h0[t].T
        L0row[R] = b_i2h0[t] + b_h2h0[t]
        L0row[R + 1: R + 1 + IN] = W_i2h0[t].T
        L1row = np.zeros((HC, GN), np.float32)
        L1row[0:R] = W_i2h1[t].T
        L1row[R] = b_i2h1[t] + b_h2h1[t]
        L1row[H1OFF: H1OFF + R] = W_h2h1[t].T

        L0hi = _q8(L0row)
        L0lo = _q8(L0row - L0hi)
        L1hi = _q8(L1row)
        L1lo = _q8(L1row - L1hi)
        planes = {"h0": L0hi, "l0": L0lo, "h1": L1hi, "l1": L1lo}

        Wt = np.zeros((NPASS, 2, 128, GNP), np.float32)
        for p, (_, _, wa, wb) in enumerate(L0P + L1P):
            for gsel, (plane, ck) in ((0, wa), (1, wb)):
                Wt[p, gsel, :, 0:GN] = planes[plane][ck * 128:(ck + 1) * 128]
        # -> [128, NPASS*2*GNP]
        w_dev[t] = np.ascontiguousarray(
            Wt.transpose(2, 0, 1, 3).reshape(128, NPASS * 2 * GNP)).astype(F8)

    init4 = init.reshape(B, 4, R)
    h0_full, c0_full = init4[:, 0], init4[:, 1]
    h1_full, c1_full = init4[:, 2], init4[:, 3]

    in_maps = []
    for c in range(NCORES):
        sl = slice(c * BC, (c + 1) * BC)
        hcp = np.zeros((BC, HC), np.float32)
        hcp[:, 0:R] = h0_full[sl]
        hcp[:, R] = 1.0
        hcp[:, R + 1: R + 1 + IN] = x[sl]
        hcp[:, H1OFF: H1OFF + R] = h1_full[sl]
        hcp = hcp.astype(BF16)
        in_maps.append({
            "w": w_dev,
            "hci": _pack_pf(hcp),
            "htci": _pack_kt(hcp),
            "c0i": _pack_pf(np.ascontiguousarray(c0_full[sl])).astype(BF16),
            "c1i": _pack_pf(np.ascontiguousarray(c1_full[sl])).astype(BF16),
        })
    return in_maps, h1_full


def kernel(x, init_states_input, W_i2h0, b_i2h0, W_h2h0, b_h2h0,
           W_i2h1, b_i2h1, W_h2h1, b_h2h1):
    global LAST_RESULT
    from concourse.bass_utils import run_bass_kernel_spmd

    in_maps, h1_full = prep_inputs(
        x, init_states_input, W_i2h0, b_i2h0, W_h2h0, b_h2h0,
        W_i2h1, b_i2h1, W_h2h1, b_h2h1)

    nc = build_bass(NSTEPS)
    res = run_bass_kernel_spmd(nc, in_maps, list(range(NCORES)), trace=TRACE)
    LAST_RESULT = res

    out = np.empty((B, (NSTEPS + 1) * R), np.float32)
    out[:, 0:R] = h1_full
    for c in range(NCORES):
        # device out[p, m, :] = batch row m*128+p
        dev = np.asarray(res.results[c]["out"], np.float32)
        out[c * BC:(c + 1) * BC, R:] = (
            dev.transpose(1, 0, 2).reshape(BC, NSTEPS * R))
    return out


# revision 58
# speedup vs baseline: 1.4288x; 1.0145x over previous
"""Bass/Trainium2 kernel for nn_BuildLstmUnrollNet — fp8 DoubleRow version.

Problem: 2-layer LSTM, unrolled T=11 steps with per-step (non-shared)
weights, B=8192, R=425, IN=20.  Output block t is the last-layer h
*before* step t, so only steps 0..9 need computing.

Strategy (data-parallel over batch, 8 cores x 1024 rows):
  - Gates are computed batch-major with the *transposed activations*
    stationary (lhsT) in fp8e4 and the weights moving in fp8e4, using
    perf_mode=DoubleRow (2 fp8 MACs/cell/cycle -> 2x PE throughput).
  - Error compensation to stay inside the 2e-2 gate:
      * weights split hi+lo (two fp8 planes, effective ~11-bit weights)
      * activations: fp8-hi everywhere plus fp8-lo "compensation rows"
        for 256 of the 425 h rows per tensor (h0, h0', h1), carried as
        extra K-chunks whose weight blocks are the hi planes.
    Per m-tile per step: layer0 = 5 DoubleRow passes, layer1 = 9.
  - The recurrent transpose h -> hT runs directly SBUF->SBUF on the
    xbar (bf16), no DRAM bounce; DVE converts the transposed bf16 to
    the fp8 hi plane and computes the fp8 lo plane (hi-lo subtract).
  - Cell math in bf16 on DVE (2x mode), c kept bf16; one fused sigmoid
    over i|f|o + tanh(g) straight out of PSUM on ACT; tanh(c) merged
    over m-tile pairs.

kernel(**inputs) takes full-size numpy inputs, does the host-side
packing/sharding, runs the same program SPMD on cores 0..7, and
reassembles the full [8192, 4675] fp32 output (block 0 comes straight
from the initial state on the host).
"""

import numpy as np
import ml_dtypes

BF16 = ml_dtypes.bfloat16
F8 = ml_dtypes.float8_e4m3fn

B = 8192
NCORES = 8
BC = B // NCORES          # batch rows per core (1024)
NB = BC // 128            # m-tiles per core (8)
R = 425
IN = 20
GN = 4 * R                # 1700 gate columns
GNP = 1712                # 16-aligned weight group stride
H1OFF = R + 1 + IN        # 446: h1 col offset in the packed state block
HC = 896                  # packed state block width (7*128)
NKC = HC // 128           # 7 hi chunks
NCH = 11                  # 7 hi + 4 lo chunks (7=lo c0, 8=lo c1, 9=lo c4, 10=lo c5)
NSTEPS = 10
NCHUNKS_A = [(0, 512), (512, 512), (1024, 251)]   # i|f|o -> tile A
NCHUNKS_B = [(1275, 425)]                         # g gate -> tile B

# Pass plans: (chunkA, chunkB, wsrcA, wsrcB); wsrc = (plane, chunk) with
# plane in {h0,l0} (layer-0 hi/lo) or {h1,l1} (layer-1 hi/lo).
L0P = [
    (0, 1, ("h0", 0), ("h0", 1)),
    (2, 3, ("h0", 2), ("h0", 3)),
    (0, 1, ("l0", 0), ("l0", 1)),
    (2, 3, ("l0", 2), ("l0", 3)),
]
# h0'-only passes first, h1-dependent chunks (3 is mid-step, 4..6,9,10 are
# end-of-previous-step) last, so layer-1 matmuls start before the h1
# fp8 planes for this step are finished
L1P = [
    (0, 1, ("h1", 0), ("h1", 1)),
    (2, 3, ("h1", 2), ("h1", 3)),
    (0, 1, ("l1", 0), ("l1", 1)),
    (2, 3, ("l1", 2), ("l1", 3)),
    (6, 7, ("h1", 6), ("h1", 0)),      # + act-comp h0' rows 0..127
    (6, 8, ("l1", 6), ("h1", 1)),      # + act-comp h0' rows 128..255
    (4, 5, ("h1", 4), ("h1", 5)),
    (4, 5, ("l1", 4), ("l1", 5)),
    (9, 10, ("h1", 4), ("h1", 5)),     # act-comp h1 rows 66..321
]
NPASS = len(L0P) + len(L1P)            # 14

# set by test.py to profile; results stashed in LAST_RESULT
TRACE = False
LAST_RESULT = None


def build_bass(n_steps=NSTEPS, finalize=True):
    import concourse.bacc as bacc
    import concourse.mybir as mybir
    import concourse.tile as tile

    f32 = mybir.dt.float32
    bf16 = mybir.dt.bfloat16
    f8 = mybir.dt.float8e4
    DR = mybir.MatmulPerfMode.DoubleRow
    Sig = mybir.ActivationFunctionType.Sigmoid
    Tanh = mybir.ActivationFunctionType.Tanh

    nc = bacc.Bacc()

    w_d = nc.declare_dram_parameter("w", [n_steps, 128, NPASS * 2 * GNP], f8,
                                    False)
    hci_d = nc.declare_dram_parameter("hci", [128, NB * HC], bf16, False)
    htci_d = nc.declare_dram_parameter("htci", [128, NKC * BC], bf16, False)
    c0i_d = nc.declare_dram_parameter("c0i", [128, NB * R], bf16, False)
    c1i_d = nc.declare_dram_parameter("c1i", [128, NB * R], bf16, False)
    # out[p, m, t*R+r] = h1 block t for batch row m*128+p (host reassembles)
    out_d = nc.declare_dram_parameter("out", [128, NB, n_steps * R], bf16, True)

    with tile.TileContext(nc) as tc:
        with (
            tc.tile_pool(name="consts", bufs=1) as consts,
            tc.tile_pool(name="wpool", bufs=2) as wpool,
            tc.tile_pool(name="gpsum", bufs=2, space="PSUM") as gpsum,
            tc.tile_pool(name="tmp", bufs=6) as tmp,
            tc.tile_pool(name="h1pool", bufs=2) as h1pool,
        ):
            # persistent state tiles
            hs_t = consts.tile([128, NB * HC], bf16)   # packed batch-major
            htc = consts.tile([128, NKC, BC], bf16)    # transposed, bf16
            htc8 = consts.tile([128, NCH, BC], f8)     # fp8 hi + lo chunks
            c0 = consts.tile([128, NB * R], bf16)
            c1 = consts.tile([128, NB * R], bf16)

            # init DMAs on the SP (HWDGE) queue while the first weight
            # chunks stream on the Pool (SWDGE) queue
            for k in range(NKC):
                nc.sync.dma_start(htc[:, k, :], htci_d[:, k * BC:(k + 1) * BC])
            nc.sync.dma_start(c0[:], c0i_d[:])
            nc.sync.dma_start(hs_t[:], hci_d[:])
            nc.sync.dma_start(c1[:], c1i_d[:])

            # step-0 weights, split per pass-pair so matmuls start early
            w = wpool.tile([128, NPASS * 2, GNP], f8, tag="w")
            for p in range(NPASS):
                nc.gpsimd.dma_start(
                    w[:, 2 * p: 2 * p + 2, :],
                    w_d[0][:, 2 * p * GNP: (2 * p + 2) * GNP])

            # initial fp8 conversion of the transposed state
            nc.vector.tensor_copy(htc8[:, 0:NKC, :], htc[:, 0:NKC, :])
            nc.vector.tensor_sub(htc8[:, 7:9, :], htc[:, 0:2, :],
                                 htc8[:, 0:2, :])
            nc.vector.tensor_sub(htc8[:, 9:11, :], htc[:, 4:6, :],
                                 htc8[:, 4:6, :])

            # PE warm-up: the HAM clock gate needs ~3.4us of sustained
            # activity before the PE runs at full rate.
            warm = consts.tile([128, 128], bf16)
            nc.vector.memset(warm[:], 0.0)
            wps = gpsum.tile([128, 3 * R], f32, tag="ga")
            for i in range(20):
                nc.tensor.matmul(wps[:, 0:128], warm[:], warm[:],
                                 start=True, stop=True)



            for t in range(n_steps):
                h1f = h1pool.tile([128, NB, R], bf16, tag="h1f")
                if t > 0:
                    # h1 chunks (4..6) were transposed at the end of step
                    # t-1; produce their fp8 hi planes + h1 lo chunks on
                    # GPSIMD so neither the DVE nor the ACT in-order
                    # queues ever wait on them.  Layer 1's h1 passes (the
                    # last 5 of L1P) are the only consumers.  These go
                    # FIRST on the Pool queue: the w prefetch below has a
                    # WAR wait on step t-1's matmuls and a step of slack.
                    nc.gpsimd.tensor_copy(htc8[:, 4:7, :], htc[:, 4:7, :])
                    nc.gpsimd.tensor_sub(htc8[:, 9:11, :], htc[:, 4:6, :],
                                         htc8[:, 4:6, :])

                stash = {L: {"osig": [None] * NB, "ottc": [None] * NB}
                         for L in (0, 1)}

                def run_cell(layer, m):
                    passes = L0P if layer == 0 else L1P
                    poff = 0 if layer == 0 else len(L0P)
                    nk = len(passes)
                    cst = c0 if layer == 0 else c1
                    osig = stash[layer]["osig"]
                    ottc = stash[layer]["ottc"]

                    def finishA(m, cst):
                        """tanh(c), merged over m-tile pairs (c is
                        contiguous across m) — emitted after cell m+1's
                        sig/tanh so the ACT queue never waits on DVE."""
                        if m % 2 == 0:
                            return          # emitted with its pair partner
                        cs = cst[:, (m - 1) * R:(m + 1) * R]
                        ttc = tmp.tile([128, 2 * R], bf16, tag="ttc")
                        nc.scalar.activation(ttc[:], cs, Tanh)
                        ottc[m - 1] = ttc[:, 0:R]
                        ottc[m] = ttc[:, R:2 * R]

                    def finishB(m, layer, t):
                        """h-mul + transpose/fp8 for cell m — trails by 2
                        cells so the DVE queue never waits on tanh(c)."""
                        tsig = osig[m]
                        ttc = ottc[m]
                        if layer == 0:
                            nc.vector.tensor_mul(
                                hs_t[:, m * HC: m * HC + R],
                                tsig[:, 2 * R:3 * R], ttc[:])
                        else:
                            hh = h1f[:, m, :]
                            nc.vector.tensor_mul(hh, tsig[:, 2 * R:3 * R],
                                                 ttc[:])
                            if t < n_steps - 1:
                                nc.vector.tensor_copy(
                                    hs_t[:, m * HC + H1OFF:
                                         m * HC + H1OFF + R], hh)
                        cols = slice(m * 128, (m + 1) * 128)
                        if layer == 0:
                            # transpose THIS m-tile's h0'|1|x|h1head cols,
                            # then its fp8 hi planes + h0' lo chunks —
                            # layer 1's m-tile m only reads its own 128
                            # columns, so it unblocks immediately
                            nc.sync.dma_start(
                                htc[:, 0:4, cols],
                                hs_t[:, m * HC: m * HC + 512],
                                transpose=True)
                            eng = nc.vector if m < 2 else nc.gpsimd
                            eng.tensor_copy(htc8[:, 0:4, cols],
                                            htc[:, 0:4, cols])
                            eng.tensor_sub(htc8[:, 7:9, cols],
                                           htc[:, 0:2, cols],
                                           htc8[:, 0:2, cols])
                        elif t < n_steps - 1:
                            # this m-tile's h1 cols (chunks 4..6); their
                            # fp8 planes are made at the top of step t+1
                            nc.sync.dma_start(
                                htc[:, 4:7, cols],
                                hs_t[:, m * HC + 512:(m + 1) * HC],
                                transpose=True)

                    if True:
                        g_a = gpsum.tile([128, 3 * R], f32, tag="ga")
                        g_b = gpsum.tile([128, R], f32, tag="gb")
                        for ki, (ca, cb, _, _) in enumerate(passes):
                            st = cb - ca
                            lhsT = htc8[:, ca:cb + 1:st,
                                        m * 128:(m + 1) * 128]
                            p = poff + ki
                            for (no, nw) in NCHUNKS_A:
                                nc.tensor.matmul(
                                    g_a[:, no: no + nw],
                                    lhsT,
                                    w[:, 2 * p: 2 * p + 2, no: no + nw],
                                    start=(ki == 0),
                                    stop=(ki == nk - 1),
                                    perf_mode=DR,
                                )
                            for (no, nw) in NCHUNKS_B:
                                nc.tensor.matmul(
                                    g_b[:, no - 3 * R: no - 3 * R + nw],
                                    lhsT,
                                    w[:, 2 * p: 2 * p + 2, no: no + nw],
                                    start=(ki == 0),
                                    stop=(ki == nk - 1),
                                    perf_mode=DR,
                                )

                        # cell part A (torch gate order: i, f, o, g), bf16
                        cs = cst[:, m * R:(m + 1) * R]
                        tsig = tmp.tile([128, 3 * R], bf16, tag="tsig")
                        nc.scalar.activation(tsig[:], g_a[:, 0:3 * R], Sig)
                        tg = tmp.tile([128, R], bf16, tag="tg")
                        nc.scalar.activation(tg[:], g_b[:, 0:R], Tanh)
                        tig = tmp.tile([128, R], bf16, tag="tig")
                        nc.vector.tensor_mul(tig[:], tsig[:, 0:R], tg[:])
                        tfc = tmp.tile([128, R], bf16, tag="tfc")
                        nc.vector.tensor_mul(tfc[:], tsig[:, R:2 * R], cs)
                        nc.vector.tensor_add(cs, tfc[:], tig[:])
                        osig[m] = tsig

                        if m >= 1:
                            finishA(m - 1, cst)
                        if m >= 2:
                            finishB(m - 2, layer, t)
                        if m == NB - 1:
                            finishA(NB - 1, cst)
                            finishB(NB - 2, layer, t)
                            finishB(NB - 1, layer, t)

                # interleave the ACT-bound L0 phase (4 passes/cell) with
                # the PE-bound L1 phase (9 passes/cell): L1-m needs L0-m's
                # transposed+converted h0', ready ~2 cells after L0-m, so
                # stagger L1 five cells behind
                ORDER = [(0, 0), (0, 1), (0, 2), (0, 3), (0, 4),
                         (0, 5), (0, 6), (1, 0), (0, 7), (1, 1),
                         (1, 2), (1, 3), (1, 4), (1, 5), (1, 6), (1, 7)]
                for (L, m) in ORDER:
                    run_cell(L, m)

                # output store per step on the SP queue, two pieces
                nc.sync.dma_start(
                    out_d[:, 0:4, t * R:(t + 1) * R], h1f[:, 0:4, :])
                nc.sync.dma_start(
                    out_d[:, 4:8, t * R:(t + 1) * R], h1f[:, 4:8, :])
                if t < n_steps - 1:
                    # prefetch step t+1's weights (a full step of slack;
                    # emitted last so the Pool queue serves the mid-step
                    # fp8 converts first)
                    w_next = wpool.tile([128, NPASS * 2, GNP], f8, tag="w")
                    for c in range(4):
                        glo = c * 7
                        ghi = min((c + 1) * 7, 2 * NPASS)
                        nc.gpsimd.dma_start(
                            w_next[:, glo: ghi, :],
                            w_d[t + 1][:, glo * GNP: ghi * GNP])
                    w = w_next
    if finalize:
        nc.finalize()
    return nc


def _pack_pf(a):
    """[BC, C] -> [128, NB*C] with m-tile m at cols m*C."""
    c = a.shape[1]
    return np.ascontiguousarray(
        a.reshape(NB, 128, c).transpose(1, 0, 2).reshape(128, NB * c))


def _pack_kt(a):
    """[BC, HC] -> transposed [128, NKC*BC] with K-chunk k at cols k*BC."""
    return np.ascontiguousarray(
        a.T.reshape(NKC, 128, BC).transpose(1, 0, 2).reshape(128, NKC * BC))


def _q8(a):
    return a.astype(F8).astype(np.float32)


def prep_inputs(x, init_states_input, W_i2h0, b_i2h0, W_h2h0, b_h2h0,
                W_i2h1, b_i2h1, W_h2h1, b_h2h1, n_steps=NSTEPS):
    """Host-side packing.  Returns (in_maps, h1_init_full)."""
    x = np.asarray(x, np.float32)
    init = np.asarray(init_states_input, np.float32)
    W_i2h0 = np.asarray(W_i2h0, np.float32)
    b_i2h0 = np.asarray(b_i2h0, np.float32)
    W_h2h0 = np.asarray(W_h2h0, np.float32)
    b_h2h0 = np.asarray(b_h2h0, np.float32)
    W_i2h1 = np.asarray(W_i2h1, np.float32)
    b_i2h1 = np.asarray(b_i2h1, np.float32)
    W_h2h1 = np.asarray(W_h2h1, np.float32)
    b_h2h1 = np.asarray(b_h2h1, np.float32)

    # per-step packed-row weight planes, then per-pass fp8 blocks
    w_dev = np.zeros((n_steps, 128, NPASS * 2 * GNP), F8)
    for t in range(n_steps):
        L0row = np.zeros((512, GN), np.float32)
        L0row[0:R] = W_h2h0[t].T
        L0row[R] = b_i2h0[t] + b_h2h0[t]
        L0row[R + 1: R + 1 + IN] = W_i2h0[t].T
        L1row = np.zeros((HC, GN), np.float32)
        L1row[0:R] = W_i2h1[t].T
        L1row[R] = b_i2h1[t] + b_h2h1[t]
        L1row[H1OFF: H1OFF + R] = W_h2h1[t].T

        L0hi = _q8(L0row)
        L0lo = _q8(L0row - L0hi)
        L1hi = _q8(L1row)
        L1lo = _q8(L1row - L1hi)
        planes = {"h0": L0hi, "l0": L0lo, "h1": L1hi, "l1": L1lo}

        Wt = np.zeros((NPASS, 2, 128, GNP), np.float32)
        for p, (_, _, wa, wb) in enumerate(L0P + L1P):
            for gsel, (plane, ck) in ((0, wa), (1, wb)):
                Wt[p, gsel, :, 0:GN] = planes[plane][ck * 128:(ck + 1) * 128]
        # -> [128, NPASS*2*GNP]
        w_dev[t] = np.ascontiguousarray(
            Wt.transpose(2, 0, 1, 3).reshape(128, NPASS * 2 * GNP)).astype(F8)

    init4 = init.reshape(B, 4, R)
    h0_full, c0_full = init4[:, 0], init4[:, 1]
    h1_full, c1_full = init4[:, 2], init4[:, 3]

    in_maps = []
    for c in range(NCORES):
        sl = slice(c * BC, (c + 1) * BC)
        hcp = np.zeros((BC, HC), np.float32)
        hcp[:, 0:R] = h0_full[sl]
        hcp[:, R] = 1.0
        hcp[:, R + 1: R + 1 + IN] = x[sl]
        hcp[:, H1OFF: H1OFF + R] = h1_full[sl]
        hcp = hcp.astype(BF16)
        in_maps.append({
            "w": w_dev,
            "hci": _pack_pf(hcp),
            "htci": _pack_kt(hcp),
            "c0i": _pack_pf(np.ascontiguousarray(c0_full[sl])).astype(BF16),
            "c1i": _pack_pf(np.ascontiguousarray(c1_full[sl])).astype(BF16),
        })
    return in_maps, h1_full


def kernel(x, init_states_input, W_i2h0, b_i2h0, W_h2h0, b_h2h0,
           W_i2h1, b_i2h1, W_h2h1, b_h2h1):
    global LAST_RESULT
    from concourse.bass_utils import run_bass_kernel_spmd

    in_maps, h1_full = prep_inputs(
        x, init_states_input, W_i2h0, b_i2h0, W_h2h0, b_h2h0,
        W_i2h1, b_i2h1, W_h2h1, b_h2h1)

    nc = build_bass(NSTEPS)
    res = run_bass_kernel_spmd(nc, in_maps, list(range(NCORES)), trace=TRACE)
    LAST_RESULT = res

    out = np.empty((B, (NSTEPS + 1) * R), np.float32)
    out[:, 0:R] = h1_full
    for c in range(NCORES):
        # device out[p, m, :] = batch row m*128+p
        dev = np.asarray(res.results[c]["out"], np.float32)
        out[c * BC:(c + 1) * BC, R:] = (
            dev.transpose(1, 0, 2).reshape(BC, NSTEPS * R))
    return out


# revision 64
# speedup vs baseline: 1.4292x; 1.0003x over previous
"""Bass/Trainium2 kernel for nn_BuildLstmUnrollNet — fp8 DoubleRow version.

Problem: 2-layer LSTM, unrolled T=11 steps with per-step (non-shared)
weights, B=8192, R=425, IN=20.  Output block t is the last-layer h
*before* step t, so only steps 0..9 need computing.

Strategy (data-parallel over batch, 8 cores x 1024 rows):
  - Gates are computed batch-major with the *transposed activations*
    stationary (lhsT) in fp8e4 and the weights moving in fp8e4, using
    perf_mode=DoubleRow (2 fp8 MACs/cell/cycle -> 2x PE throughput).
  - Error compensation to stay inside the 2e-2 gate:
      * weights split hi+lo (two fp8 planes, effective ~11-bit weights)
      * activations: fp8-hi everywhere plus fp8-lo "compensation rows"
        for 256 of the 425 h rows per tensor (h0, h0', h1), carried as
        extra K-chunks whose weight blocks are the hi planes.
    Per m-tile per step: layer0 = 5 DoubleRow passes, layer1 = 9.
  - The recurrent transpose h -> hT runs directly SBUF->SBUF on the
    xbar (bf16), no DRAM bounce; DVE converts the transposed bf16 to
    the fp8 hi plane and computes the fp8 lo plane (hi-lo subtract).
  - Cell math in bf16 on DVE (2x mode), c kept bf16; one fused sigmoid
    over i|f|o + tanh(g) straight out of PSUM on ACT; tanh(c) merged
    over m-tile pairs.

kernel(**inputs) takes full-size numpy inputs, does the host-side
packing/sharding, runs the same program SPMD on cores 0..7, and
reassembles the full [8192, 4675] fp32 output (block 0 comes straight
from the initial state on the host).
"""

import numpy as np
import ml_dtypes

BF16 = ml_dtypes.bfloat16
F8 = ml_dtypes.float8_e4m3fn

B = 8192
NCORES = 8
BC = B // NCORES          # batch rows per core (1024)
NB = BC // 128            # m-tiles per core (8)
R = 425
IN = 20
GN = 4 * R                # 1700 gate columns
GNP = 1712                # 16-aligned weight group stride
H1OFF = R + 1 + IN        # 446: h1 col offset in the packed state block
HC = 896                  # packed state block width (7*128)
NKC = HC // 128           # 7 hi chunks
NCH = 11                  # 7 hi + 4 lo chunks (7=lo c0, 8=lo c1, 9=lo c4, 10=lo c5)
NSTEPS = 10
NCHUNKS_A = [(0, 512), (512, 512), (1024, 251)]   # i|f|o -> tile A
NCHUNKS_B = [(1275, 425)]                         # g gate -> tile B

# Pass plans: (chunkA, chunkB, wsrcA, wsrcB); wsrc = (plane, chunk) with
# plane in {h0,l0} (layer-0 hi/lo) or {h1,l1} (layer-1 hi/lo).
L0P = [
    (0, 1, ("h0", 0), ("h0", 1)),
    (2, 3, ("h0", 2), ("h0", 3)),
    (0, 1, ("l0", 0), ("l0", 1)),
    (2, 3, ("l0", 2), ("l0", 3)),
]
# h0'-only passes first, h1-dependent chunks (3 is mid-step, 4..6,9,10 are
# end-of-previous-step) last, so layer-1 matmuls start before the h1
# fp8 planes for this step are finished
L1P = [
    (0, 1, ("h1", 0), ("h1", 1)),
    (2, 3, ("h1", 2), ("h1", 3)),
    (0, 1, ("l1", 0), ("l1", 1)),
    (2, 3, ("l1", 2), ("l1", 3)),
    (6, 7, ("h1", 6), ("h1", 0)),      # + act-comp h0' rows 0..127
    (6, 8, ("l1", 6), ("h1", 1)),      # + act-comp h0' rows 128..255
    (4, 5, ("h1", 4), ("h1", 5)),
    (4, 5, ("l1", 4), ("l1", 5)),
    (9, 10, ("h1", 4), ("h1", 5)),     # act-comp h1 rows 66..321
]
NPASS = len(L0P) + len(L1P)            # 14

# set by test.py to profile; results stashed in LAST_RESULT
TRACE = False
LAST_RESULT = None


def build_bass(n_steps=NSTEPS, finalize=True):
    import concourse.bacc as bacc
    import concourse.mybir as mybir
    import concourse.tile as tile

    f32 = mybir.dt.float32
    bf16 = mybir.dt.bfloat16
    f8 = mybir.dt.float8e4
    DR = mybir.MatmulPerfMode.DoubleRow
    Sig = mybir.ActivationFunctionType.Sigmoid
    Tanh = mybir.ActivationFunctionType.Tanh

    nc = bacc.Bacc()

    w_d = nc.declare_dram_parameter("w", [n_steps, 128, NPASS * 2 * GNP], f8,
                                    False)
    hci_d = nc.declare_dram_parameter("hci", [128, NB * HC], bf16, False)
    htci_d = nc.declare_dram_parameter("htci", [128, NKC * BC], bf16, False)
    c0i_d = nc.declare_dram_parameter("c0i", [128, NB * R], bf16, False)
    c1i_d = nc.declare_dram_parameter("c1i", [128, NB * R], bf16, False)
    # out[p, m, t*R+r] = h1 block t for batch row m*128+p (host reassembles)
    out_d = nc.declare_dram_parameter("out", [128, NB, n_steps * R], bf16, True)

    with tile.TileContext(nc) as tc:
        with (
            tc.tile_pool(name="consts", bufs=1) as consts,
            tc.tile_pool(name="wpool", bufs=2) as wpool,
            tc.tile_pool(name="gpsum", bufs=2, space="PSUM") as gpsum,
            tc.tile_pool(name="tmp", bufs=6) as tmp,
            tc.tile_pool(name="h1pool", bufs=2) as h1pool,
        ):
            # persistent state tiles
            hs_t = consts.tile([128, NB * HC], bf16)   # packed batch-major
            htc = consts.tile([128, NKC, BC], bf16)    # transposed, bf16
            htc8 = consts.tile([128, NCH, BC], f8)     # fp8 hi + lo chunks
            c0 = consts.tile([128, NB * R], bf16)
            c1 = consts.tile([128, NB * R], bf16)

            # init DMAs on the SP (HWDGE) queue while the first weight
            # chunks stream on the Pool (SWDGE) queue
            for k in range(NKC):
                nc.sync.dma_start(htc[:, k, :], htci_d[:, k * BC:(k + 1) * BC])
            nc.sync.dma_start(c0[:], c0i_d[:])
            nc.sync.dma_start(hs_t[:], hci_d[:])
            nc.sync.dma_start(c1[:], c1i_d[:])

            # step-0 weights, split per pass-pair so matmuls start early
            w = wpool.tile([128, NPASS * 2, GNP], f8, tag="w")
            for p in range(NPASS):
                nc.gpsimd.dma_start(
                    w[:, 2 * p: 2 * p + 2, :],
                    w_d[0][:, 2 * p * GNP: (2 * p + 2) * GNP])

            # initial fp8 conversion of the transposed state
            nc.vector.tensor_copy(htc8[:, 0:NKC, :], htc[:, 0:NKC, :])
            nc.vector.tensor_sub(htc8[:, 7:9, :], htc[:, 0:2, :],
                                 htc8[:, 0:2, :])
            nc.vector.tensor_sub(htc8[:, 9:11, :], htc[:, 4:6, :],
                                 htc8[:, 4:6, :])

            # PE warm-up: the HAM clock gate needs ~3.4us of sustained
            # activity before the PE runs at full rate.
            warm = consts.tile([128, 128], bf16)
            nc.vector.memset(warm[:], 0.0)
            wps = gpsum.tile([128, 3 * R], f32, tag="ga")
            for i in range(12):
                nc.tensor.matmul(wps[:, 0:128], warm[:], warm[:],
                                 start=True, stop=True)



            for t in range(n_steps):
                h1f = h1pool.tile([128, NB, R], bf16, tag="h1f")
                if t > 0:
                    # h1 chunks (4..6) were transposed at the end of step
                    # t-1; produce their fp8 hi planes + h1 lo chunks on
                    # GPSIMD so neither the DVE nor the ACT in-order
                    # queues ever wait on them.  Layer 1's h1 passes (the
                    # last 5 of L1P) are the only consumers.  These go
                    # FIRST on the Pool queue: the w prefetch below has a
                    # WAR wait on step t-1's matmuls and a step of slack.
                    nc.gpsimd.tensor_copy(htc8[:, 4:7, :], htc[:, 4:7, :])
                    nc.gpsimd.tensor_sub(htc8[:, 9:11, :], htc[:, 4:6, :],
                                         htc8[:, 4:6, :])

                stash = {L: {"osig": [None] * NB, "ottc": [None] * NB}
                         for L in (0, 1)}

                def run_cell(layer, m):
                    passes = L0P if layer == 0 else L1P
                    poff = 0 if layer == 0 else len(L0P)
                    nk = len(passes)
                    cst = c0 if layer == 0 else c1
                    osig = stash[layer]["osig"]
                    ottc = stash[layer]["ottc"]

                    def finishA(m, cst):
                        """tanh(c), merged over m-tile pairs (c is
                        contiguous across m) — emitted after cell m+1's
                        sig/tanh so the ACT queue never waits on DVE."""
                        if m % 2 == 0:
                            return          # emitted with its pair partner
                        cs = cst[:, (m - 1) * R:(m + 1) * R]
                        ttc = tmp.tile([128, 2 * R], bf16, tag="ttc")
                        nc.scalar.activation(ttc[:], cs, Tanh)
                        ottc[m - 1] = ttc[:, 0:R]
                        ottc[m] = ttc[:, R:2 * R]

                    def finishB(m, layer, t):
                        """h-mul + transpose/fp8 for cell m — trails by 2
                        cells so the DVE queue never waits on tanh(c)."""
                        tsig = osig[m]
                        ttc = ottc[m]
                        if layer == 0:
                            nc.vector.tensor_mul(
                                hs_t[:, m * HC: m * HC + R],
                                tsig[:, 2 * R:3 * R], ttc[:])
                        else:
                            hh = h1f[:, m, :]
                            nc.vector.tensor_mul(hh, tsig[:, 2 * R:3 * R],
                                                 ttc[:])
                            if t < n_steps - 1:
                                nc.vector.tensor_copy(
                                    hs_t[:, m * HC + H1OFF:
                                         m * HC + H1OFF + R], hh)
                        cols = slice(m * 128, (m + 1) * 128)
                        if layer == 0:
                            # transpose THIS m-tile's h0'|1|x|h1head cols,
                            # then its fp8 hi planes + h0' lo chunks —
                            # layer 1's m-tile m only reads its own 128
                            # columns, so it unblocks immediately
                            nc.sync.dma_start(
                                htc[:, 0:4, cols],
                                hs_t[:, m * HC: m * HC + 512],
                                transpose=True)
                            eng = nc.vector if m < 2 else nc.gpsimd
                            eng.tensor_copy(htc8[:, 0:4, cols],
                                            htc[:, 0:4, cols])
                            eng.tensor_sub(htc8[:, 7:9, cols],
                                           htc[:, 0:2, cols],
                                           htc8[:, 0:2, cols])
                        elif t < n_steps - 1:
                            # this m-tile's h1 cols (chunks 4..6); their
                            # fp8 planes are made at the top of step t+1
                            nc.sync.dma_start(
                                htc[:, 4:7, cols],
                                hs_t[:, m * HC + 512:(m + 1) * HC],
                                transpose=True)

                    if True:
                        g_a = gpsum.tile([128, 3 * R], f32, tag="ga")
                        g_b = gpsum.tile([128, R], f32, tag="gb")
                        for ki, (ca, cb, _, _) in enumerate(passes):
                            st = cb - ca
                            lhsT = htc8[:, ca:cb + 1:st,
                                        m * 128:(m + 1) * 128]
                            p = poff + ki
                            for (no, nw) in NCHUNKS_A:
                                nc.tensor.matmul(
                                    g_a[:, no: no + nw],
                                    lhsT,
                                    w[:, 2 * p: 2 * p + 2, no: no + nw],
                                    start=(ki == 0),
                                    stop=(ki == nk - 1),
                                    perf_mode=DR,
                                )
                            for (no, nw) in NCHUNKS_B:
                                nc.tensor.matmul(
                                    g_b[:, no - 3 * R: no - 3 * R + nw],
                                    lhsT,
                                    w[:, 2 * p: 2 * p + 2, no: no + nw],
                                    start=(ki == 0),
                                    stop=(ki == nk - 1),
                                    perf_mode=DR,
                                )

                        # cell part A (torch gate order: i, f, o, g), bf16
                        cs = cst[:, m * R:(m + 1) * R]
                        tsig = tmp.tile([128, 3 * R], bf16, tag="tsig")
                        nc.scalar.activation(tsig[:], g_a[:, 0:3 * R], Sig)
                        tg = tmp.tile([128, R], bf16, tag="tg")
                        nc.scalar.activation(tg[:], g_b[:, 0:R], Tanh)
                        tig = tmp.tile([128, R], bf16, tag="tig")
                        nc.vector.tensor_mul(tig[:], tsig[:, 0:R], tg[:])
                        tfc = tmp.tile([128, R], bf16, tag="tfc")
                        nc.vector.tensor_mul(tfc[:], tsig[:, R:2 * R], cs)
                        nc.vector.tensor_add(cs, tfc[:], tig[:])
                        osig[m] = tsig

                        if m >= 1:
                            finishA(m - 1, cst)
                        if m >= 2:
                            finishB(m - 2, layer, t)
                        if m == NB - 1:
                            finishA(NB - 1, cst)
                            finishB(NB - 2, layer, t)
                            finishB(NB - 1, layer, t)

                # interleave the ACT-bound L0 phase (4 passes/cell) with
                # the PE-bound L1 phase (9 passes/cell): L1-m needs L0-m's
                # transposed+converted h0', ready ~2 cells after L0-m, so
                # stagger L1 five cells behind
                ORDER = [(0, 0), (0, 1), (0, 2), (0, 3), (0, 4),
                         (0, 5), (0, 6), (1, 0), (1, 1), (0, 7),
                         (1, 2), (1, 3), (1, 4), (1, 5), (1, 6), (1, 7)]
                for (L, m) in ORDER:
                    run_cell(L, m)

                # output store per step on the SP queue, two pieces
                nc.sync.dma_start(
                    out_d[:, 0:4, t * R:(t + 1) * R], h1f[:, 0:4, :])
                nc.sync.dma_start(
                    out_d[:, 4:8, t * R:(t + 1) * R], h1f[:, 4:8, :])
                if t < n_steps - 1:
                    # prefetch step t+1's weights (a full step of slack;
                    # emitted last so the Pool queue serves the mid-step
                    # fp8 converts first)
                    w_next = wpool.tile([128, NPASS * 2, GNP], f8, tag="w")
                    for c in range(4):
                        glo = c * 7
                        ghi = min((c + 1) * 7, 2 * NPASS)
                        nc.gpsimd.dma_start(
                            w_next[:, glo: ghi, :],
                            w_d[t + 1][:, glo * GNP: ghi * GNP])
                    w = w_next
    if finalize:
        nc.finalize()
    return nc


def _pack_pf(a):
    """[BC, C] -> [128, NB*C] with m-tile m at cols m*C."""
    c = a.shape[1]
    return np.ascontiguousarray(
        a.reshape(NB, 128, c).transpose(1, 0, 2).reshape(128, NB * c))


def _pack_kt(a):
    """[BC, HC] -> transposed [128, NKC*BC] with K-chunk k at cols k*BC."""
    return np.ascontiguousarray(
        a.T.reshape(NKC, 128, BC).transpose(1, 0, 2).reshape(128, NKC * BC))


def _q8(a):
    return a.astype(F8).astype(np.float32)


def prep_inputs(x, init_states_input, W_i2h0, b_i2h0, W_h2h0, b_h2h0,
                W_i2h1, b_i2h1, W_h2h1, b_h2h1, n_steps=NSTEPS):
    """Host-side packing.  Returns (in_maps, h1_init_full)."""
    x = np.asarray(x, np.float32)
    init = np.asarray(init_states_input, np.float32)
    W_i2h0 = np.asarray(W_i2h0, np.float32)
    b_i2h0 = np.asarray(b_i2h0, np.float32)
    W_h2h0 = np.asarray(W_h2h0, np.float32)
    b_h2h0 = np.asarray(b_h2h0, np.float32)
    W_i2h1 = np.asarray(W_i2h1, np.float32)
    b_i2h1 = np.asarray(b_i2h1, np.float32)
    W_h2h1 = np.asarray(W_h2h1, np.float32)
    b_h2h1 = np.asarray(b_h2h1, np.float32)

    # per-step packed-row weight planes, then per-pass fp8 blocks
    w_dev = np.zeros((n_steps, 128, NPASS * 2 * GNP), F8)
    for t in range(n_steps):
        L0row = np.zeros((512, GN), np.float32)
        L0row[0:R] = W_h2h0[t].T
        L0row[R] = b_i2h0[t] + b_h2h0[t]
        L0row[R + 1: R + 1 + IN] = W_i2h0[t].T
        L1row = np.zeros((HC, GN), np.float32)
        L1row[0:R] = W_i2h1[t].T
        L1row[R] = b_i2h1[t] + b_h2h1[t]
        L1row[H1OFF: H1OFF + R] = W_h2h1[t].T

        L0hi = _q8(L0row)
        L0lo = _q8(L0row - L0hi)
        L1hi = _q8(L1row)
        L1lo = _q8(L1row - L1hi)
        planes = {"h0": L0hi, "l0": L0lo, "h1": L1hi, "l1": L1lo}

        Wt = np.zeros((NPASS, 2, 128, GNP), np.float32)
        for p, (_, _, wa, wb) in enumerate(L0P + L1P):
            for gsel, (plane, ck) in ((0, wa), (1, wb)):
                Wt[p, gsel, :, 0:GN] = planes[plane][ck * 128:(ck + 1) * 128]
        # -> [128, NPASS*2*GNP]
        w_dev[t] = np.ascontiguousarray(
            Wt.transpose(2, 0, 1, 3).reshape(128, NPASS * 2 * GNP)).astype(F8)

    init4 = init.reshape(B, 4, R)
    h0_full, c0_full = init4[:, 0], init4[:, 1]
    h1_full, c1_full = init4[:, 2], init4[:, 3]

    in_maps = []
    for c in range(NCORES):
        sl = slice(c * BC, (c + 1) * BC)
        hcp = np.zeros((BC, HC), np.float32)
        hcp[:, 0:R] = h0_full[sl]
        hcp[:, R] = 1.0
        hcp[:, R + 1: R + 1 + IN] = x[sl]
        hcp[:, H1OFF: H1OFF + R] = h1_full[sl]
        hcp = hcp.astype(BF16)
        in_maps.append({
            "w": w_dev,
            "hci": _pack_pf(hcp),
            "htci": _pack_kt(hcp),
            "c0i": _pack_pf(np.ascontiguousarray(c0_full[sl])).astype(BF16),
            "c1i": _pack_pf(np.ascontiguousarray(c1_full[sl])).astype(BF16),
        })
    return in_maps, h1_full


def kernel(x, init_states_input, W_i2h0, b_i2h0, W_h2h0, b_h2h0,
           W_i2h1, b_i2h1, W_h2h1, b_h2h1):
    global LAST_RESULT
    from concourse.bass_utils import run_bass_kernel_spmd

    in_maps, h1_full = prep_inputs(
        x, init_states_input, W_i2h0, b_i2h0, W_h2h0, b_h2h0,
        W_i2h1, b_i2h1, W_h2h1, b_h2h1)

    nc = build_bass(NSTEPS)
    res = run_bass_kernel_spmd(nc, in_maps, list(range(NCORES)), trace=TRACE)
    LAST_RESULT = res

    out = np.empty((B, (NSTEPS + 1) * R), np.float32)
    out[:, 0:R] = h1_full
    for c in range(NCORES):
        # device out[p, m, :] = batch row m*128+p
        dev = np.asarray(res.results[c]["out"], np.float32)
        out[c * BC:(c + 1) * BC, R:] = (
            dev.transpose(1, 0, 2).reshape(BC, NSTEPS * R))
    return out
